# revision 1
# baseline (speedup 1.0000x reference)
"""Trainium2 Bass kernel for nn_Decoder_46042049413334.

Buggy 2-layer LSTM decoder with attention (B=32, T=64, S=128, D=512).

Structure (per core, batch sharded 8 ways, BS=4 examples/core):
  Phase A: xpart0 = [emb(tokens), 1] @ [W_ih0.T; b0]   (batched over all t)
  Pass  B: layer-0 recurrence over t (only W_hh0 streamed per step)
  Phase C: h2_0 = sigma_o * tanh(c2) batched; xpart1 = [h2_0, 1] @ [W_ih1.T; b1]
  Pass  D: layer-1 recurrence over t (only W_hh1 streamed per step)
  Phase E: s = sigma_o1 * tanh(c2_1); attention + out-projection (batched)

Weight gate-columns are permuted so each 512-wide N-block of the gates psum is
a complete {i,f,o,g} set for a 128-wide d-block (sub-order i,f,o,g), letting
the nonlinearity pipeline against the PE weight stream. Matmuls run in bf16
(PSUM accumulation fp32); the per-step critical chain is sigma(i,f) ->
tanh(g) -> c2 -> transpose -> bf16 copy, with sigma(o) deferred.

Row ordering is b-major everywhere: row r = b_local*T + t.
"""
import numpy as np
import ml_dtypes
from contextlib import ExitStack

import concourse.bass as bass
import concourse.bacc as bacc
import concourse.tile as tile
from concourse import mybir, masks
from concourse.bass_utils import run_bass_kernel_spmd

F32 = mybir.dt.float32
BF16 = mybir.dt.bfloat16
AF = mybir.ActivationFunctionType
NPBF = ml_dtypes.bfloat16

B, T, S, D, L, V = 32, 64, 128, 512, 2, 32000
G = 4 * D        # 2048
DS = 2 * D       # 1024
NCORES = 8
BS = B // NCORES  # 4
R = BS * T        # 256 rows per core


# ---------------------------------------------------------------- host side

def _gate_perm():
    perm = np.zeros(G, dtype=np.int64)
    base = {0: 0, 1: 512, 2: 1536, 3: 1024}  # i, f, o, g
    for j in range(G):
        nb, pos = divmod(j, 512)
        sub, dd = divmod(pos, 128)
        perm[j] = base[sub] + nb * 128 + dd
    return perm


def host_prep(inputs):
    """Build the 8 per-core input maps (layout/gather work only)."""
    perm = _gate_perm()
    tokens = np.asarray(inputs["prev_tgt_tokens"])
    embed = np.asarray(inputs["embed"], dtype=np.float32)
    enc = np.asarray(inputs["encoder_out"], dtype=np.float32)
    mask = np.asarray(inputs["src_mask"])
    hid = np.asarray(inputs["hiddens"], dtype=np.float32)
    cells = np.asarray(inputs["cells"], dtype=np.float32)
    W_ih = np.asarray(inputs["W_ih"], dtype=np.float32)
    W_hh = np.asarray(inputs["W_hh"], dtype=np.float32)
    b_ih = np.asarray(inputs["b_ih"], dtype=np.float32)
    b_hh = np.asarray(inputs["b_hh"], dtype=np.float32)
    W_in = np.asarray(inputs["W_in"], dtype=np.float32)
    b_in = np.asarray(inputs["b_in"], dtype=np.float32)
    W_out = np.asarray(inputs["W_out"], dtype=np.float32)
    b_out = np.asarray(inputs["b_out"], dtype=np.float32)

    def bf(x):
        return np.ascontiguousarray(x, dtype=NPBF)

    WIH = []
    WHH = []
    for l in range(L):
        wihT = W_ih[l].T[:, perm]
        biasrow = (b_ih[l] + b_hh[l])[perm][None, :]
        WIH.append(bf(np.concatenate([wihT, biasrow], 0)))   # [513, 2048]
        WHH.append(bf(W_hh[l].T[:, perm]))                   # [512, 2048]
    WINT = bf(W_in.T)                                        # [512, 1024]
    WOUTT = bf(np.concatenate([W_out.T, b_out[None, :]], 0))  # [1537, 512]

    in_maps = []
    for core in range(NCORES):
        bsl = slice(core * BS, (core + 1) * BS)
        xe = embed[tokens[bsl]]                              # [BS, T, D]
        Xaug = np.concatenate(
            [xe.reshape(R, D), np.ones((R, 1), np.float32)], axis=1)
        XT0 = bf(Xaug.T)                                     # [513, 256]
        enc_c = np.ascontiguousarray(enc[bsl])               # [BS, 128, 1024]
        encT_c = np.swapaxes(enc_c, 1, 2)                    # [BS, 1024, 128]
        offs = np.einsum("bsd,d->bs", enc_c, b_in) + np.where(mask[bsl], -1e9, 0.0)
        offs_rep = np.ascontiguousarray(
            np.broadcast_to(offs[:, None, :], (BS, T, S)), dtype=np.float32)
        hidT = np.swapaxes(hid[:, bsl], 1, 2)                # [L, D, BS]
        # pair layout for initial c2T: [L, pair, p, 36] cols {0:4, 32:36}
        h5 = hidT.reshape(L, 2, 2, 128, BS).transpose(0, 1, 3, 2, 4)
        hidTp = np.zeros((L, 2, 128, 36), np.float32)
        hidTp[..., 0:BS] = h5[:, :, :, 0, :]
        hidTp[..., 32:32 + BS] = h5[:, :, :, 1, :]
        in_maps.append({
            "xt0": XT0,
            "wih0": WIH[0], "whh0": WHH[0],
            "wih1": WIH[1], "whh1": WHH[1],
            "wint": WINT, "woutt": WOUTT,
            "enc": bf(enc_c), "enct": bf(encT_c), "offs": offs_rep,
            "hidt": bf(hidTp), "cells": np.ascontiguousarray(cells[:, bsl]),
            "ones1": np.ones((1, R), NPBF),
            "id4": np.eye(BS, dtype=NPBF),
        })
    return in_maps


# ------------------------------------------------------------- device build

def build_program():
    nc = bacc.Bacc("TRN2", target_bir_lowering=False, debug=False)

    XT0 = nc.dram_tensor("xt0", [513, R], BF16, kind="ExternalInput")
    WIH0 = nc.dram_tensor("wih0", [513, G], BF16, kind="ExternalInput")
    WHH0 = nc.dram_tensor("whh0", [D, G], BF16, kind="ExternalInput")
    WIH1 = nc.dram_tensor("wih1", [513, G], BF16, kind="ExternalInput")
    WHH1 = nc.dram_tensor("whh1", [D, G], BF16, kind="ExternalInput")
    WINT = nc.dram_tensor("wint", [D, DS], BF16, kind="ExternalInput")
    WOUTT = nc.dram_tensor("woutt", [DS + D + 1, D], BF16, kind="ExternalInput")
    ENC = nc.dram_tensor("enc", [BS, S, DS], BF16, kind="ExternalInput")
    ENCT = nc.dram_tensor("enct", [BS, DS, S], BF16, kind="ExternalInput")
    OFFS = nc.dram_tensor("offs", [BS, T, S], F32, kind="ExternalInput")
    HIDT = nc.dram_tensor("hidt", [L, 2, 128, 36], BF16, kind="ExternalInput")
    CELLS = nc.dram_tensor("cells", [L, BS, D], F32, kind="ExternalInput")
    ONES1 = nc.dram_tensor("ones1", [1, R], BF16, kind="ExternalInput")
    ID4 = nc.dram_tensor("id4", [BS, BS], BF16, kind="ExternalInput")
    OUT = nc.dram_tensor("out", [BS, T, D], F32, kind="ExternalOutput")

    XP0 = nc.dram_tensor("xp0", [BS, T, G], BF16, kind="Internal")
    XP1 = nc.dram_tensor("xp1", [BS, T, G], BF16, kind="Internal")
    H2S = nc.dram_tensor("h2s", [L, BS, T, D], F32, kind="Internal")

    with tile.TileContext(nc) as tc, ExitStack() as ctx:
        cpool = ctx.enter_context(tc.tile_pool(name="const", bufs=1))
        ident = cpool.tile([128, 128], F32)
        masks.make_identity(nc, ident[:])
        ones = cpool.tile([1, R], BF16)
        nc.sync.dma_start(ones[:], ONES1.ap())
        i4r = cpool.tile([BS, BS], BF16)
        nc.sync.dma_start(i4r[:], ID4.ap())

        psp = ctx.enter_context(tc.tile_pool(name="ps", bufs=1, space="PSUM"))

        def gtile(idx, shape):
            return psp.tile(shape, F32, tag=f"g{idx}", name=f"g{idx}",
                            bufs=2 if idx < 3 else 1)

        def batched_xpart(wpool, lhs_tiles, W_dram, XP_dram):
            """xpart = lhsT.T @ W  -> XP_dram (bf16)."""
            wt = [wpool.tile([128, G], BF16, tag=f"wk{k}", name=f"wk{k}")
                  for k in range(4)]
            wt.append(wpool.tile([1, G], BF16, tag="wk4", name="wk4"))
            for k in range(4):
                nc.sync.dma_start(wt[k][:], W_dram.ap()[128 * k:128 * (k + 1), :])
            nc.sync.dma_start(wt[4][:], W_dram.ap()[512:513, :])
            xpflat = XP_dram.ap().rearrange("b t g -> (b t) g")
            for mc in range(2):
                for nb in range(4):
                    ps = gtile(nb, [128, 512])
                    for k in range(5):
                        nc.tensor.matmul(
                            ps[:],
                            lhs_tiles[k][:, 128 * mc:128 * (mc + 1)],
                            wt[k][:, 512 * nb:512 * (nb + 1)],
                            start=(k == 0), stop=(k == 4))
                    sb = wpool.tile([128, 512], BF16, tag=f"stg{nb}",
                                    name=f"stg{nb}")
                    nc.scalar.copy(sb[:], ps[:])
                    nc.sync.dma_start(
                        xpflat[128 * mc:128 * (mc + 1), 512 * nb:512 * (nb + 1)],
                        sb[:])

        # hoisted W_hh loads for both layers (overlap with phase A)
        wbpool = ctx.enter_context(tc.tile_pool(name="wb", bufs=1))
        whh_all = {}
        for l, Wd in ((0, WHH0), (1, WHH1)):
            tiles = [wbpool.tile([128, G], BF16, tag=f"whh{l}k{k}",
                                 name=f"whh{l}k{k}") for k in range(4)]
            for k in range(4):
                nc.sync.dma_start(
                    tiles[k][:], Wd.ap()[128 * k:128 * (k + 1), :])
            whh_all[l] = tiles

        # ---------------- Phase A: xpart0 ----------------
        with tc.tile_pool(name="pa", bufs=1) as pa:
            xt = [pa.tile([128, R], BF16, tag=f"xt{k}", name=f"xt{k}")
                  for k in range(4)]
            xt.append(pa.tile([1, R], BF16, tag="xt4", name="xt4"))
            for k in range(4):
                nc.sync.dma_start(xt[k][:], XT0.ap()[128 * k:128 * (k + 1), :])
            nc.sync.dma_start(xt[4][:], XT0.ap()[512:513, :])
            batched_xpart(pa, xt, WIH0, XP0)

        # ---------------- Recurrence passes ----------------
        def recur(l, WHH_dram, XP_dram):
            with tc.tile_pool(name=f"pb{l}", bufs=1) as pb, \
                 tc.tile_pool(name=f"pd{l}", bufs=3) as pd:
                whh = whh_all[l]
                cl = pb.tile([BS, D], F32, tag="cells", name="cells")
                nc.sync.dma_start(cl[:], CELLS.ap()[l])
                c2T = []
                for pair in range(2):
                    tl = pd.tile([128, 36], BF16, tag=f"c2Tp{pair}",
                                 name=f"c2Tp{pair}")
                    nc.sync.dma_start(tl[:], HIDT.ap()[l, pair])
                    c2T.append(tl)

                for t in range(T):
                    xp = pd.tile([BS, G], BF16, tag="xp", name="xp")
                    nc.sync.dma_start(xp[:], XP_dram.ap()[:, t, :])
                    c2T_new = [None] * 2
                    c2h = [None] * 2
                    gps = []
                    sgs = []
                    # per-block: xpart via K=BS identity matmul + 4 W chunks
                    for nb in range(4):
                        nsl = slice(512 * nb, 512 * (nb + 1))
                        pair, sub = divmod(nb, 2)
                        ps = gtile(nb, [BS, 512])
                        gps.append(ps)
                        nc.tensor.matmul(ps[:], i4r[:], xp[:, nsl],
                                         start=True, stop=False)
                        for k in range(4):
                            nc.tensor.matmul(
                                ps[:], c2T[k // 2][:, 32 * (k % 2):32 * (k % 2) + 4],
                                whh[k][:, nsl], start=False, stop=(k == 3))
                        # critical chain: sigma(i,f), tanh(g), c2 block
                        sg = pd.tile([BS, 256], F32, tag=f"sg{nb}",
                                     name=f"sg{nb}")
                        nc.scalar.activation(sg[:], ps[:, 0:256], AF.Sigmoid)
                        sgs.append(sg)
                        tg = pd.tile([BS, 128], F32, tag=f"tg{nb}",
                                     name=f"tg{nb}")
                        nc.scalar.activation(tg[:], ps[:, 384:512], AF.Tanh)
                        t1 = pd.tile([BS, 128], F32, tag=f"t1{nb}",
                                     name=f"t1{nb}")
                        nc.vector.tensor_mul(
                            t1[:], sg[:, 128:256],
                            cl[:, 128 * nb:128 * (nb + 1)])
                        t2 = pd.tile([BS, 128], F32, tag=f"t2{nb}",
                                     name=f"t2{nb}")
                        nc.vector.tensor_mul(t2[:], sg[:, 0:128], tg[:])
                        if sub == 0:
                            c2h[pair] = pd.tile([36, 128], F32,
                                                tag=f"c2h{pair}",
                                                name=f"c2h{pair}")
                        nc.vector.tensor_add(
                            c2h[pair][32 * sub:32 * sub + 4, :], t1[:], t2[:])
                        if sub == 1:
                            # one stacked transpose per block pair
                            tp = psp.tile([128, 36], F32,
                                          tag="tp0", name="tp0")
                            nc.tensor.transpose(
                                tp[:], c2h[pair][:], ident[0:36, 0:36])
                            nt = pd.tile([128, 36], BF16,
                                         tag=f"c2Tp{pair}", name=f"c2Tp{pair}")
                            nc.vector.tensor_copy(nt[:], tp[:])
                            c2T_new[pair] = nt
                    # deferred: h2 = sigma(o) * tanh(c2) -> H2S rows
                    h2row = pd.tile([BS, D], F32, tag="h2row", name="h2row")
                    for nb in range(4):
                        pair, sub = divmod(nb, 2)
                        so = pd.tile([BS, 128], F32, tag=f"so{nb}",
                                     name=f"so{nb}")
                        nc.scalar.activation(
                            so[:], gps[nb][:, 256:384], AF.Sigmoid)
                        tc2 = pd.tile([BS, 128], F32, tag=f"tc2{nb}",
                                      name=f"tc2{nb}")
                        nc.scalar.activation(
                            tc2[:], c2h[pair][32 * sub:32 * sub + 4, :],
                            AF.Tanh)
                        nc.vector.tensor_mul(
                            h2row[:, 128 * nb:128 * (nb + 1)], so[:], tc2[:])
                    nc.sync.dma_start(H2S.ap()[l, :, t, :], h2row[:])
                    c2T = c2T_new

        recur(0, WHH0, XP0)

        # ---------------- Phase C: h2_0 batched; xpart1 ----------------
        def rows_from_stores(pool, l, tagpfx):
            """Load 2 tiles [128, 512] f32 of h2/s rows (b-major)."""
            flat = H2S.ap()[l].rearrange("b t d -> (b t) d")
            outt = []
            for mc in range(2):
                msl = slice(128 * mc, 128 * (mc + 1))
                h2 = pool.tile([128, D], F32, tag=f"{tagpfx}h{mc}",
                               name=f"{tagpfx}h{mc}")
                nc.sync.dma_start(h2[:], flat[msl, :])
                outt.append(h2)
            return outt

        def transpose_rows(pool, rows, tagpfx):
            """rows: 2 tiles [128, 512] f32 -> 4 bf16 tiles [128, 256] (T)."""
            tT = [pool.tile([128, R], BF16, tag=f"{tagpfx}T{k}",
                            name=f"{tagpfx}T{k}") for k in range(4)]
            for mc in range(2):
                for k in range(4):
                    tp = psp.tile([128, 128], F32, tag="tp0", name="tp0")
                    nc.tensor.transpose(
                        tp[:], rows[mc][:, 128 * k:128 * (k + 1)], ident[:])
                    if k % 2 == 0:
                        nc.scalar.copy(tT[k][:, 128 * mc:128 * (mc + 1)], tp[:])
                    else:
                        nc.vector.tensor_copy(
                            tT[k][:, 128 * mc:128 * (mc + 1)], tp[:])
            return tT

        with tc.tile_pool(name="pc", bufs=1) as pc:
            h2rows = rows_from_stores(pc, 0, "h")
            h2T = transpose_rows(pc, h2rows, "h")
            lhs = h2T + [ones]
            batched_xpart(pc, lhs, WIH1, XP1)

        recur(1, WHH1, XP1)

        # ---------------- Phase E: attention + out proj ----------------
        with tc.tile_pool(name="pe", bufs=1) as pe:
            srows = rows_from_stores(pe, 1, "s")
            sT = transpose_rows(pe, srows, "s")

            wint = [pe.tile([128, DS], BF16, tag=f"wi{k}", name=f"wi{k}")
                    for k in range(4)]
            for k in range(4):
                nc.sync.dma_start(wint[k][:], WINT.ap()[128 * k:128 * (k + 1), :])
            xqT = []
            for m in range(8):
                ps = gtile(m % 4, [128, R])
                for k in range(4):
                    nc.tensor.matmul(
                        ps[:], wint[k][:, 128 * m:128 * (m + 1)], sT[k][:],
                        start=(k == 0), stop=(k == 3))
                xq = pe.tile([128, R], BF16, tag=f"xq{m}", name=f"xq{m}")
                if m % 2 == 0:
                    nc.scalar.copy(xq[:], ps[:])
                else:
                    nc.vector.tensor_copy(xq[:], ps[:])
                xqT.append(xq)

            ctxT = [pe.tile([128, R], BF16, tag=f"cx{m}", name=f"cx{m}")
                    for m in range(8)]
            for b in range(BS):
                bsl = slice(T * b, T * (b + 1))
                encb = pe.tile([S, DS], BF16, tag=f"enc{b}", name=f"enc{b}")
                nc.sync.dma_start(encb[:], ENC.ap()[b])
                enctb = [pe.tile([128, S], BF16, tag=f"ect{b}{k}",
                                 name=f"ect{b}{k}") for k in range(8)]
                for k in range(8):
                    nc.sync.dma_start(
                        enctb[k][:], ENCT.ap()[b, 128 * k:128 * (k + 1), :])
                eps = gtile(2 + (b % 2), [T, S])
                for k in range(8):
                    nc.tensor.matmul(
                        eps[:], xqT[k][:, bsl], enctb[k][:],
                        start=(k == 0), stop=(k == 7))
                offsb = pe.tile([T, S], F32, tag="offs", name="offs")
                nc.sync.dma_start(offsb[:], OFFS.ap()[b])
                esb = pe.tile([T, S], F32, tag="esb", name="esb")
                nc.vector.tensor_add(esb[:], eps[:], offsb[:])
                negmax = pe.tile([T, 1], F32, tag="negmax", name="negmax")
                nc.vector.reduce_max(
                    negmax[:], esb[:], axis=mybir.AxisListType.X, negate=True)
                expE = pe.tile([T, S], F32, tag="expE", name="expE")
                den = pe.tile([T, 1], F32, tag="den", name="den")
                nc.scalar.activation(
                    expE[:], esb[:], AF.Exp, bias=negmax[:], accum_out=den[:])
                rden = pe.tile([T, 1], F32, tag="rden", name="rden")
                nc.vector.reciprocal(rden[:], den[:])
                attn = pe.tile([T, S], F32, tag="attn", name="attn")
                nc.vector.tensor_scalar_mul(attn[:], expE[:], rden[:])
                tp = psp.tile([S, T], F32, tag="tp0", name="tp0")
                nc.tensor.transpose(tp[:], attn[:], ident[0:T, 0:T])
                atsb = pe.tile([S, T], BF16, tag="atsb", name="atsb")
                nc.vector.tensor_copy(atsb[:], tp[:])
                for m in range(8):
                    psc = gtile(m % 4, [128, T])
                    nc.tensor.matmul(
                        psc[:], encb[:, 128 * m:128 * (m + 1)], atsb[:],
                        start=True, stop=True)
                    if m % 2 == 0:
                        nc.scalar.copy(ctxT[m][:, bsl], psc[:])
                    else:
                        nc.vector.tensor_copy(ctxT[m][:, bsl], psc[:])

            wout = [pe.tile([128, D], BF16, tag=f"wo{k}", name=f"wo{k}")
                    for k in range(12)]
            for k in range(12):
                nc.sync.dma_start(wout[k][:], WOUTT.ap()[128 * k:128 * (k + 1), :])
            woutb = pe.tile([1, D], BF16, tag="wo12", name="wo12")
            nc.sync.dma_start(woutb[:], WOUTT.ap()[1536:1537, :])
            outflat = OUT.ap().rearrange("b t d -> (b t) d")
            lhs_all = ctxT + sT + [ones]
            wt_all = wout + [woutb]
            for mc in range(2):
                msl = slice(128 * mc, 128 * (mc + 1))
                ps = gtile(mc, [128, D])
                for k in range(13):
                    nc.tensor.matmul(
                        ps[:], lhs_all[k][:, msl], wt_all[k][:],
                        start=(k == 0), stop=(k == 12))
                osb = pe.tile([128, D], F32, tag=f"osb{mc}", name=f"osb{mc}")
                nc.scalar.activation(osb[:], ps[:], AF.Tanh)
                nc.sync.dma_start(outflat[msl, :], osb[:])

    nc.compile()
    return nc


def assemble(results):
    full = np.concatenate([r["out"] for r in results], axis=0)  # [B, T, D]
    outs = full.transpose(1, 0, 2)                              # [T, B, D]
    return np.ascontiguousarray(outs.reshape(-1, D).reshape(-1, T, D))


_nc_cache = None


def kernel(**inputs):
    global _nc_cache
    in_maps = host_prep(inputs)
    if _nc_cache is None:
        _nc_cache = build_program()
    res = run_bass_kernel_spmd(_nc_cache, in_maps, list(range(NCORES)))
    return assemble(res.results)



# revision 7
# speedup vs baseline: 1.1066x; 1.1066x over previous
"""Trainium2 Bass kernel for nn_Decoder_46042049413334.

Buggy 2-layer LSTM decoder with attention (B=32, T=64, S=128, D=512).

Structure (per core, batch sharded 8 ways, BS=4 examples/core):
  Phase A: xpart0 = [emb(tokens), 1] @ [W_ih0.T; b0]  -> XPsb (SBUF resident)
  Pass  B: layer-0 recurrence over t, 4x column-tiled PE matmuls
  Phase C: xpart1 from hT0 (transposed h2 accumulated in-loop)
  Pass  D: layer-1 recurrence
  Phase E: attention + out-projection from hT1

Recurrence layout: gates PSUM [128, 512] where partition 32*j+b holds
(example b, d-block j) and the 512 free cols are {i,f,o,g}x128 for that
d-block. The four d-blocks' weight streams run CONCURRENTLY in the PE
array via tile_position=(0, 32*j) column tiling; xpart is injected with
a single K=16 selector matmul from the SBUF-resident xpart buffer.
Elementwise ops run once over all 128 partitions (bf16), c2 and h2 are
re-transposed per step ([128,128] PE transpose); transposed h2 history
(hT) feeds phases C/E directly so no DRAM roundtrip or batch transposes
are needed.

Row ordering is b-major everywhere: row r = b_local*T + t.
"""
import numpy as np
import ml_dtypes
from contextlib import ExitStack

import concourse.bass as bass
import concourse.bacc as bacc
import concourse.tile as tile
from concourse import mybir, masks
from concourse.bass_utils import run_bass_kernel_spmd

F32 = mybir.dt.float32
BF16 = mybir.dt.bfloat16
AF = mybir.ActivationFunctionType
NPBF = ml_dtypes.bfloat16

B, T, S, D, L, V = 32, 64, 128, 512, 2, 32000
G = 4 * D        # 2048
DS = 2 * D       # 1024
NCORES = 8
BS = B // NCORES  # 4
R = BS * T        # 256 rows per core


# ---------------------------------------------------------------- host side

def _gate_perm():
    perm = np.zeros(G, dtype=np.int64)
    base = {0: 0, 1: 512, 2: 1536, 3: 1024}  # i, f, o, g
    for j in range(G):
        nb, pos = divmod(j, 512)
        sub, dd = divmod(pos, 128)
        perm[j] = base[sub] + nb * 128 + dd
    return perm


def host_prep(inputs):
    """Build the 8 per-core input maps (layout/gather work only)."""
    perm = _gate_perm()
    tokens = np.asarray(inputs["prev_tgt_tokens"])
    embed = np.asarray(inputs["embed"], dtype=np.float32)
    enc = np.asarray(inputs["encoder_out"], dtype=np.float32)
    mask = np.asarray(inputs["src_mask"])
    hid = np.asarray(inputs["hiddens"], dtype=np.float32)
    cells = np.asarray(inputs["cells"], dtype=np.float32)
    W_ih = np.asarray(inputs["W_ih"], dtype=np.float32)
    W_hh = np.asarray(inputs["W_hh"], dtype=np.float32)
    b_ih = np.asarray(inputs["b_ih"], dtype=np.float32)
    b_hh = np.asarray(inputs["b_hh"], dtype=np.float32)
    W_in = np.asarray(inputs["W_in"], dtype=np.float32)
    b_in = np.asarray(inputs["b_in"], dtype=np.float32)
    W_out = np.asarray(inputs["W_out"], dtype=np.float32)
    b_out = np.asarray(inputs["b_out"], dtype=np.float32)

    def bf(x):
        return np.ascontiguousarray(x, dtype=NPBF)

    WIH = []
    WHH = []
    for l in range(L):
        wihT = W_ih[l].T[:, perm]
        biasrow = (b_ih[l] + b_hh[l])[perm][None, :]
        WIH.append(bf(np.concatenate([wihT, biasrow], 0)))   # [513, 2048]
        WHH.append(bf(W_hh[l].T[:, perm]))                   # [512, 2048]
    WINT = bf(W_in.T)                                        # [512, 1024]
    WOUTT = bf(np.concatenate([W_out.T, b_out[None, :]], 0))  # [1537, 512]

    # K=16 xpart injection selector: E[4j+b, 32j+b] = 1
    einj = np.zeros((16, 128), np.float32)
    for j in range(4):
        for b in range(BS):
            einj[4 * j + b, 32 * j + b] = 1.0
    einj = bf(einj)

    in_maps = []
    for core in range(NCORES):
        bsl = slice(core * BS, (core + 1) * BS)
        xe = embed[tokens[bsl]]                              # [BS, T, D]
        Xaug = np.concatenate(
            [xe.reshape(R, D), np.ones((R, 1), np.float32)], axis=1)
        XT0 = bf(Xaug.T)                                     # [513, 256]
        enc_c = np.ascontiguousarray(enc[bsl])               # [BS, 128, 1024]
        encT_c = np.swapaxes(enc_c, 1, 2)                    # [BS, 1024, 128]
        offs = np.einsum("bsd,d->bs", enc_c, b_in) + np.where(mask[bsl], -1e9, 0.0)
        offs_rep = np.ascontiguousarray(
            np.broadcast_to(offs[:, None, :], (BS, T, S)), dtype=np.float32)
        # initial c2T: c2t0[l, p, 32k+b] = hid[l, b, 128k+p]
        th = hid[:, bsl].reshape(L, BS, 4, 128).transpose(0, 3, 2, 1)  # [L,128,4,BS]
        c2t0 = np.zeros((L, 128, 4, 32), np.float32)
        c2t0[:, :, :, 0:BS] = th
        c2t0 = bf(c2t0.reshape(L, 128, 128))
        # cells in partition layout: cellsp[l, 32j+b, p] = cells[l, b, 128j+p]
        tc_ = cells[:, bsl].reshape(L, BS, 4, 128).transpose(0, 2, 1, 3)  # [L,4,BS,128]
        cellsp = np.zeros((L, 4, 32, 128), np.float32)
        cellsp[:, :, 0:BS, :] = tc_
        cellsp = bf(cellsp.reshape(L, 128, 128))
        in_maps.append({
            "xt0": XT0,
            "wih0": WIH[0], "whh0": WHH[0],
            "wih1": WIH[1], "whh1": WHH[1],
            "wint": WINT, "woutt": WOUTT,
            "enc": bf(enc_c), "enct": bf(encT_c), "offs": offs_rep,
            "c2t0": c2t0, "cellsp": cellsp,
            "ones1": np.ones((1, R), NPBF),
            "einj": einj,
        })
    return in_maps


# ------------------------------------------------------------- device build

def build_program():
    nc = bacc.Bacc("TRN2", target_bir_lowering=False, debug=False)

    XT0 = nc.dram_tensor("xt0", [513, R], BF16, kind="ExternalInput")
    WIH0 = nc.dram_tensor("wih0", [513, G], BF16, kind="ExternalInput")
    WHH0 = nc.dram_tensor("whh0", [D, G], BF16, kind="ExternalInput")
    WIH1 = nc.dram_tensor("wih1", [513, G], BF16, kind="ExternalInput")
    WHH1 = nc.dram_tensor("whh1", [D, G], BF16, kind="ExternalInput")
    WINT = nc.dram_tensor("wint", [D, DS], BF16, kind="ExternalInput")
    WOUTT = nc.dram_tensor("woutt", [DS + D + 1, D], BF16, kind="ExternalInput")
    ENC = nc.dram_tensor("enc", [BS, S, DS], BF16, kind="ExternalInput")
    ENCT = nc.dram_tensor("enct", [BS, DS, S], BF16, kind="ExternalInput")
    OFFS = nc.dram_tensor("offs", [BS, T, S], F32, kind="ExternalInput")
    C2T0 = nc.dram_tensor("c2t0", [L, 128, 128], BF16, kind="ExternalInput")
    CELLSP = nc.dram_tensor("cellsp", [L, 128, 128], BF16, kind="ExternalInput")
    ONES1 = nc.dram_tensor("ones1", [1, R], BF16, kind="ExternalInput")
    EINJ = nc.dram_tensor("einj", [16, 128], BF16, kind="ExternalInput")
    OUT = nc.dram_tensor("out", [BS, T, D], F32, kind="ExternalOutput")

    with tile.TileContext(nc) as tc, ExitStack() as ctx:
        cpool = ctx.enter_context(tc.tile_pool(name="const", bufs=1))
        ident = cpool.tile([128, 128], F32)
        masks.make_identity(nc, ident[:])
        identb = cpool.tile([128, 128], BF16, name="identb")
        masks.make_identity(nc, identb[:])
        ones = cpool.tile([1, R], BF16, name="ones")
        nc.sync.dma_start(ones[:], ONES1.ap())
        einj = cpool.tile([16, 128], BF16, name="einj")
        nc.sync.dma_start(einj[:], EINJ.ap())

        psp = ctx.enter_context(tc.tile_pool(name="ps", bufs=1, space="PSUM"))

        def gtile(idx, shape):
            return psp.tile(shape, F32, tag=f"g{idx}", name=f"g{idx}",
                            bufs=2 if idx < 2 else 1)

        # persistent SBUF-resident xpart: XPsb[4*nb+b, t*512+c] = xpart[b,t,512nb+c]
        xpp = ctx.enter_context(tc.tile_pool(name="xps", bufs=1))
        XPsb = xpp.tile([16, T * 512], BF16, name="xpsb")

        # transposed h2 history per layer: hT[p, j*256 + b*64 + t]
        hTp = ctx.enter_context(tc.tile_pool(name="hT", bufs=1))
        hT = [hTp.tile([128, 4 * R], BF16, name=f"hT{l}") for l in range(L)]

        def batched_xpart(wpool, lhs_tiles, wih_tiles):
            """xpart = lhsT.T @ W -> XPsb (bf16, SBUF)."""
            for mc in range(2):
                for nb in range(4):
                    ps = gtile(nb % 2, [128, 512])
                    for k in range(5):
                        nc.tensor.matmul(
                            ps[:],
                            lhs_tiles[k][:, 128 * mc:128 * (mc + 1)],
                            wih_tiles[k][:, 512 * nb:512 * (nb + 1)],
                            start=(k == 0), stop=(k == 4))
                    sb = wpool.tile([128, 512], BF16, tag=f"stg{nb}",
                                    name=f"stg{nb}")
                    nc.scalar.copy(sb[:], ps[:])
                    dst = XPsb[4 * nb + 2 * mc:4 * nb + 2 * mc + 2, :].rearrange(
                        "p (t c) -> p t c", c=512)
                    nc.sync.dma_start(dst, sb[:])

        # ---------------- Phase A: xpart0 ----------------
        pa = ctx.enter_context(tc.tile_pool(name="pa", bufs=1))
        xt = [pa.tile([128, R], BF16, tag=f"xt{k}", name=f"xt{k}")
              for k in range(4)]
        xt.append(pa.tile([1, R], BF16, tag="xt4", name="xt4"))
        for k in range(4):
            nc.sync.dma_start(xt[k][:], XT0.ap()[128 * k:128 * (k + 1), :])
        nc.sync.dma_start(xt[4][:], XT0.ap()[512:513, :])
        wihA = [pa.tile([128, G], BF16, tag=f"wk{k}", name=f"wk{k}")
                for k in range(4)]
        wihA.append(pa.tile([1, G], BF16, tag="wk4", name="wk4"))
        for k in range(4):
            nc.sync.dma_start(wihA[k][:], WIH0.ap()[128 * k:128 * (k + 1), :])
        nc.sync.dma_start(wihA[4][:], WIH0.ap()[512:513, :])

        # prefetch pool: everything needed later, DMA-ordered by need time
        pf = ctx.enter_context(tc.tile_pool(name="pf", bufs=1))
        whh_all = {}
        for l, Wd in ((0, WHH0), (1, WHH1)):
            tiles = [pf.tile([128, G], BF16, tag=f"whh{l}k{k}",
                             name=f"whh{l}k{k}") for k in range(4)]
            for k in range(4):
                nc.sync.dma_start(
                    tiles[k][:], Wd.ap()[128 * k:128 * (k + 1), :])
            whh_all[l] = tiles
        wihC = [pf.tile([128, G], BF16, tag=f"wc{k}", name=f"wc{k}")
                for k in range(4)]
        wihC.append(pf.tile([1, G], BF16, tag="wc4", name="wc4"))
        for k in range(4):
            nc.sync.dma_start(wihC[k][:], WIH1.ap()[128 * k:128 * (k + 1), :])
        nc.sync.dma_start(wihC[4][:], WIH1.ap()[512:513, :])
        wint = [pf.tile([128, DS], BF16, tag=f"wi{k}", name=f"wi{k}")
                for k in range(4)]
        for k in range(4):
            nc.sync.dma_start(wint[k][:], WINT.ap()[128 * k:128 * (k + 1), :])
        wout = [pf.tile([128, D], BF16, tag=f"wo{k}", name=f"wo{k}")
                for k in range(12)]
        for k in range(12):
            nc.sync.dma_start(wout[k][:], WOUTT.ap()[128 * k:128 * (k + 1), :])
        woutb = pf.tile([1, D], BF16, tag="wo12", name="wo12")
        nc.sync.dma_start(woutb[:], WOUTT.ap()[1536:1537, :])
        encb = [pf.tile([S, DS], BF16, tag=f"enc{b}", name=f"enc{b}")
                for b in range(BS)]
        enctb = [[pf.tile([128, S], BF16, tag=f"ect{b}{k}", name=f"ect{b}{k}")
                  for k in range(8)] for b in range(BS)]
        offsb = [pf.tile([T, S], F32, tag=f"offs{b}", name=f"offs{b}")
                 for b in range(BS)]
        for b in range(BS):
            nc.sync.dma_start(encb[b][:], ENC.ap()[b])
            for k in range(8):
                nc.sync.dma_start(
                    enctb[b][k][:], ENCT.ap()[b, 128 * k:128 * (k + 1), :])
            nc.sync.dma_start(offsb[b][:], OFFS.ap()[b])

        batched_xpart(pa, xt, wihA)

        # ---------------- Recurrence passes ----------------
        def recur(l):
            whh = whh_all[l]
            with tc.tile_pool(name=f"pb{l}", bufs=1) as pb, \
                 tc.tile_pool(name=f"pd{l}", bufs=2) as pd:
                cT = pb.tile([128, 128], BF16, tag="cT", name="cT")
                nc.sync.dma_start(cT[:], CELLSP.ap()[l])
                c2T = pd.tile([128, 128], BF16, tag="c2T", name="c2T")
                nc.sync.dma_start(c2T[:], C2T0.ap()[l])

                def emit_h2_tail(h2t, tprev):
                    tp2 = psp.tile([128, 128], BF16, tag="tp2", name="tp2")
                    nc.tensor.transpose(tp2[:], h2t[:], identb[:])
                    src = tp2[:].rearrange("p (k r) -> p k r", k=4)[:, :, 0:BS]
                    dst = hT[l][:].rearrange(
                        "p (k b t) -> p k b t", k=4, b=BS)[:, :, :, tprev]
                    nc.vector.tensor_copy(dst, src)

                h2_prev = None
                for t in range(T):
                    gates = gtile(0, [128, 512])
                    # xpart injection: one K=16 selector matmul
                    nc.tensor.matmul(
                        gates[:], einj[:],
                        XPsb[:, 512 * t:512 * (t + 1)],
                        start=True, stop=False, skip_group_check=True)
                    # 4x column-tiled W_hh streams, accumulate over k
                    for k in range(4):
                        lhsT = c2T[:, 32 * k:32 * k + BS]
                        for j in range(4):
                            nc.tensor.matmul(
                                gates[32 * j:32 * j + BS, :],
                                lhsT, whh[k][:, 512 * j:512 * (j + 1)],
                                start=False, stop=(k == 3),
                                tile_position=(0, 32 * j),
                                skip_group_check=True)
                    sif = pd.tile([128, 256], BF16, tag="sif", name="sif")
                    nc.scalar.activation(sif[:], gates[:, 0:256], AF.Sigmoid)
                    tg = pd.tile([128, 128], BF16, tag="tg", name="tg")
                    nc.scalar.activation(tg[:], gates[:, 384:512], AF.Tanh)
                    m1 = pd.tile([128, 128], BF16, tag="m1", name="m1")
                    nc.vector.tensor_mul(m1[:], sif[:, 128:256], cT[:])
                    m2 = pd.tile([128, 128], BF16, tag="m2", name="m2")
                    nc.vector.tensor_mul(m2[:], sif[:, 0:128], tg[:])
                    c2h = pd.tile([128, 128], BF16, tag="c2h", name="c2h")
                    nc.vector.tensor_add(c2h[:], m1[:], m2[:])
                    tp = psp.tile([128, 128], BF16, tag="tp", name="tp")
                    nc.tensor.transpose(tp[:], c2h[:], identb[:])
                    c2T_new = pd.tile([128, 128], BF16, tag="c2T", name="c2T")
                    nc.vector.tensor_copy(c2T_new[:], tp[:])
                    # h2 branch (off the c2 critical chain)
                    so = pd.tile([128, 128], BF16, tag="so", name="so")
                    nc.scalar.activation(so[:], gates[:, 256:384], AF.Sigmoid)
                    tc2 = pd.tile([128, 128], BF16, tag="tc2", name="tc2")
                    nc.scalar.activation(tc2[:], c2h[:], AF.Tanh)
                    h2 = pd.tile([128, 128], BF16, tag="h2", name="h2")
                    nc.vector.tensor_mul(h2[:], so[:], tc2[:])
                    if h2_prev is not None:
                        emit_h2_tail(h2_prev, t - 1)
                    h2_prev = h2
                    c2T = c2T_new
                emit_h2_tail(h2_prev, T - 1)

        recur(0)

        # ---------------- Phase C: xpart1 from hT0 ----------------
        with tc.tile_pool(name="pc", bufs=1) as pc:
            lhs = [hT[0][:, 256 * k:256 * (k + 1)] for k in range(4)] + [ones]
            batched_xpart(pc, lhs, wihC)

        recur(1)

        # ---------------- Phase E: attention + out proj ----------------
        with tc.tile_pool(name="pe", bufs=1) as pe:
            sT = [hT[1][:, 256 * k:256 * (k + 1)] for k in range(4)]

            xqT = []
            for m in range(8):
                ps = gtile(m % 2, [128, R])
                for k in range(4):
                    nc.tensor.matmul(
                        ps[:], wint[k][:, 128 * m:128 * (m + 1)], sT[k],
                        start=(k == 0), stop=(k == 3))
                xq = pe.tile([128, R], BF16, tag=f"xq{m}", name=f"xq{m}")
                if m % 2 == 0:
                    nc.scalar.copy(xq[:], ps[:])
                else:
                    nc.vector.tensor_copy(xq[:], ps[:])
                xqT.append(xq)

            ctxT = [pe.tile([128, R], BF16, tag=f"cx{m}", name=f"cx{m}")
                    for m in range(8)]
            for b in range(BS):
                bsl = slice(T * b, T * (b + 1))
                eps = gtile(2 + (b % 2), [T, S])
                for k in range(8):
                    nc.tensor.matmul(
                        eps[:], xqT[k][:, bsl], enctb[b][k][:],
                        start=(k == 0), stop=(k == 7))
                esb = pe.tile([T, S], F32, tag=f"esb{b % 2}",
                              name=f"esb{b % 2}")
                nc.vector.tensor_add(esb[:], eps[:], offsb[b][:])
                negmax = pe.tile([T, 1], F32, tag=f"negmax{b % 2}",
                                 name=f"negmax{b % 2}")
                nc.vector.reduce_max(
                    negmax[:], esb[:], axis=mybir.AxisListType.X, negate=True)
                expE = pe.tile([T, S], F32, tag=f"expE{b % 2}",
                               name=f"expE{b % 2}")
                den = pe.tile([T, 1], F32, tag=f"den{b % 2}",
                              name=f"den{b % 2}")
                nc.scalar.activation(
                    expE[:], esb[:], AF.Exp, bias=negmax[:], accum_out=den[:])
                rden = pe.tile([T, 1], F32, tag=f"rden{b % 2}",
                               name=f"rden{b % 2}")
                nc.vector.reciprocal(rden[:], den[:])
                attn = pe.tile([T, S], F32, tag=f"attn{b % 2}",
                               name=f"attn{b % 2}")
                nc.vector.tensor_scalar_mul(attn[:], expE[:], rden[:])
                tpa = psp.tile([S, T], F32, tag="tp", name="tp")
                nc.tensor.transpose(tpa[:], attn[:], ident[0:T, 0:T])
                atsb = pe.tile([S, T], BF16, tag=f"atsb{b % 2}",
                               name=f"atsb{b % 2}")
                nc.vector.tensor_copy(atsb[:], tpa[:])
                for m in range(8):
                    psc = gtile(m % 2, [128, T])
                    nc.tensor.matmul(
                        psc[:], encb[b][:, 128 * m:128 * (m + 1)], atsb[:],
                        start=True, stop=True)
                    if m % 2 == 0:
                        nc.scalar.copy(ctxT[m][:, bsl], psc[:])
                    else:
                        nc.vector.tensor_copy(ctxT[m][:, bsl], psc[:])

            outflat = OUT.ap().rearrange("b t d -> (b t) d")
            lhs_all = ctxT + sT + [ones]
            wt_all = wout + [woutb]
            for mc in range(2):
                msl = slice(128 * mc, 128 * (mc + 1))
                ps = gtile(mc, [128, D])
                for k in range(13):
                    nc.tensor.matmul(
                        ps[:], lhs_all[k][:, msl], wt_all[k][:],
                        start=(k == 0), stop=(k == 12))
                osb = pe.tile([128, D], F32, tag=f"osb{mc}", name=f"osb{mc}")
                nc.scalar.activation(osb[:], ps[:], AF.Tanh)
                nc.sync.dma_start(outflat[msl, :], osb[:])

    nc.compile()
    return nc


def assemble(results):
    full = np.concatenate([r["out"] for r in results], axis=0)  # [B, T, D]
    outs = full.transpose(1, 0, 2)                              # [T, B, D]
    return np.ascontiguousarray(outs.reshape(-1, D).reshape(-1, T, D))


_nc_cache = None


def kernel(**inputs):
    global _nc_cache
    in_maps = host_prep(inputs)
    if _nc_cache is None:
        _nc_cache = build_program()
    res = run_bass_kernel_spmd(_nc_cache, in_maps, list(range(NCORES)))
    return assemble(res.results)


# revision 10
# speedup vs baseline: 1.2768x; 1.1539x over previous
"""Trainium2 Bass kernel for nn_Decoder_46042049413334.

Buggy 2-layer LSTM decoder with attention (B=32, T=64, S=128, D=512).

Structure (per core, batch sharded 8 ways, BS=4 examples/core):
  Phase A: xpart0 = [emb(tokens), 1] @ [W_ih0.T; b0]  -> XPsb0 (SBUF)
  Interleaved pass: layer-0 step t, layer-1 step t-LAG, and 8-step
    xpart1 chunks all in flight together, keeping the PE busy enough
    to hold the HAM clock gate open (K=8/8).
  Phase E: attention + out-projection from hT1

Recurrence layout: gates PSUM [128, 512] where partition 32*j+b holds
(example b, d-block j) and the 512 free cols are {i,f,o,g}x128 for that
d-block. The four d-blocks' weight streams run CONCURRENTLY in the PE
array via tile_position=(0, 32*j) column tiling; xpart is injected with
a single K=16 selector matmul. Elementwise ops run once over all 128
partitions (bf16); c2 and h2 are re-transposed per step ([128,128] PE
transpose). Transposed h2 history (hT) feeds the xpart1 chunks and
phase E directly. xpart1 flows through a 16-slot SBUF ring.

Row ordering is b-major everywhere: row r = b_local*T + t.
"""
import numpy as np
import ml_dtypes
from contextlib import ExitStack

import concourse.bass as bass
import concourse.bacc as bacc
import concourse.tile as tile
from concourse import mybir, masks
from concourse.bass_utils import run_bass_kernel_spmd

F32 = mybir.dt.float32
BF16 = mybir.dt.bfloat16
AF = mybir.ActivationFunctionType
NPBF = ml_dtypes.bfloat16

B, T, S, D, L, V = 32, 64, 128, 512, 2, 32000
G = 4 * D        # 2048
DS = 2 * D       # 1024
NCORES = 8
BS = B // NCORES  # 4
R = BS * T        # 256 rows per core
LAG = 11         # layer-1 recurrence lag behind layer 0
RING = 16        # xpart1 ring slots
CH = 8           # xpart1 chunk size (steps)


# ---------------------------------------------------------------- host side

def _gate_perm():
    perm = np.zeros(G, dtype=np.int64)
    base = {0: 0, 1: 512, 2: 1536, 3: 1024}  # i, f, o, g
    for j in range(G):
        nb, pos = divmod(j, 512)
        sub, dd = divmod(pos, 128)
        perm[j] = base[sub] + nb * 128 + dd
    return perm


def host_prep(inputs):
    """Build the 8 per-core input maps (layout/gather work only)."""
    perm = _gate_perm()
    tokens = np.asarray(inputs["prev_tgt_tokens"])
    embed = np.asarray(inputs["embed"], dtype=np.float32)
    enc = np.asarray(inputs["encoder_out"], dtype=np.float32)
    mask = np.asarray(inputs["src_mask"])
    hid = np.asarray(inputs["hiddens"], dtype=np.float32)
    cells = np.asarray(inputs["cells"], dtype=np.float32)
    W_ih = np.asarray(inputs["W_ih"], dtype=np.float32)
    W_hh = np.asarray(inputs["W_hh"], dtype=np.float32)
    b_ih = np.asarray(inputs["b_ih"], dtype=np.float32)
    b_hh = np.asarray(inputs["b_hh"], dtype=np.float32)
    W_in = np.asarray(inputs["W_in"], dtype=np.float32)
    b_in = np.asarray(inputs["b_in"], dtype=np.float32)
    W_out = np.asarray(inputs["W_out"], dtype=np.float32)
    b_out = np.asarray(inputs["b_out"], dtype=np.float32)

    def bf(x):
        return np.ascontiguousarray(x, dtype=NPBF)

    WIH = []
    WHH = []
    for l in range(L):
        wihT = W_ih[l].T[:, perm]
        biasrow = (b_ih[l] + b_hh[l])[perm][None, :]
        WIH.append(bf(np.concatenate([wihT, biasrow], 0)))   # [513, 2048]
        WHH.append(bf(W_hh[l].T[:, perm]))                   # [512, 2048]
    WINT = bf(W_in.T)                                        # [512, 1024]
    WOUTT = bf(np.concatenate([W_out.T, b_out[None, :]], 0))  # [1537, 512]

    # K=16 xpart injection selector: E[4j+b, 32j+b] = 1
    einj = np.zeros((16, 128), np.float32)
    for j in range(4):
        for b in range(BS):
            einj[4 * j + b, 32 * j + b] = 1.0
    einj = bf(einj)

    # L1 xpart selector: e8sel[32b+tt, 4tt+b] = 1  (tt < CH)
    e8 = np.zeros((128, 4 * CH), np.float32)
    for b in range(BS):
        for tt in range(CH):
            e8[32 * b + tt, 4 * tt + b] = 1.0
    e8 = bf(e8)

    in_maps = []
    for core in range(NCORES):
        bsl = slice(core * BS, (core + 1) * BS)
        xe = embed[tokens[bsl]]                              # [BS, T, D]
        Xaug = np.concatenate(
            [xe.reshape(R, D), np.ones((R, 1), np.float32)], axis=1)
        XT0 = bf(Xaug.T)                                     # [513, 256]
        enc_c = np.ascontiguousarray(enc[bsl])               # [BS, 128, 1024]
        encT_c = np.swapaxes(enc_c, 1, 2)                    # [BS, 1024, 128]
        offs = np.einsum("bsd,d->bs", enc_c, b_in) + np.where(mask[bsl], -1e9, 0.0)
        offs_rep = np.ascontiguousarray(
            np.broadcast_to(offs[:, None, :], (BS, T, S)), dtype=np.float32)
        # initial c2T: c2t0[l, p, 32k+b] = hid[l, b, 128k+p]
        th = hid[:, bsl].reshape(L, BS, 4, 128).transpose(0, 3, 2, 1)  # [L,128,4,BS]
        c2t0 = np.zeros((L, 128, 4, 32), np.float32)
        c2t0[:, :, :, 0:BS] = th
        c2t0 = bf(c2t0.reshape(L, 128, 128))
        # cells in partition layout: cellsp[l, 32j+b, p] = cells[l, b, 128j+p]
        tc_ = cells[:, bsl].reshape(L, BS, 4, 128).transpose(0, 2, 1, 3)  # [L,4,BS,128]
        cellsp = np.zeros((L, 4, 32, 128), np.float32)
        cellsp[:, :, 0:BS, :] = tc_
        cellsp = bf(cellsp.reshape(L, 128, 128))
        in_maps.append({
            "xt0": XT0,
            "wih0": WIH[0], "whh0": WHH[0],
            "wih1": WIH[1], "whh1": WHH[1],
            "wint": WINT, "woutt": WOUTT,
            "enc": bf(enc_c), "enct": bf(encT_c), "offs": offs_rep,
            "c2t0": c2t0, "cellsp": cellsp,
            "ones1": np.ones((1, R), NPBF),
            "einj": einj, "e8sel": e8,
        })
    return in_maps


# ------------------------------------------------------------- device build

def build_program():
    nc = bacc.Bacc("TRN2", target_bir_lowering=False, debug=False)

    XT0 = nc.dram_tensor("xt0", [513, R], BF16, kind="ExternalInput")
    WIH0 = nc.dram_tensor("wih0", [513, G], BF16, kind="ExternalInput")
    WHH0 = nc.dram_tensor("whh0", [D, G], BF16, kind="ExternalInput")
    WIH1 = nc.dram_tensor("wih1", [513, G], BF16, kind="ExternalInput")
    WHH1 = nc.dram_tensor("whh1", [D, G], BF16, kind="ExternalInput")
    WINT = nc.dram_tensor("wint", [D, DS], BF16, kind="ExternalInput")
    WOUTT = nc.dram_tensor("woutt", [DS + D + 1, D], BF16, kind="ExternalInput")
    ENC = nc.dram_tensor("enc", [BS, S, DS], BF16, kind="ExternalInput")
    ENCT = nc.dram_tensor("enct", [BS, DS, S], BF16, kind="ExternalInput")
    OFFS = nc.dram_tensor("offs", [BS, T, S], F32, kind="ExternalInput")
    C2T0 = nc.dram_tensor("c2t0", [L, 128, 128], BF16, kind="ExternalInput")
    CELLSP = nc.dram_tensor("cellsp", [L, 128, 128], BF16, kind="ExternalInput")
    ONES1 = nc.dram_tensor("ones1", [1, R], BF16, kind="ExternalInput")
    EINJ = nc.dram_tensor("einj", [16, 128], BF16, kind="ExternalInput")
    E8SEL = nc.dram_tensor("e8sel", [128, 4 * CH], BF16, kind="ExternalInput")
    OUT = nc.dram_tensor("out", [BS, T, D], F32, kind="ExternalOutput")

    with tile.TileContext(nc) as tc, ExitStack() as ctx:
        cpool = ctx.enter_context(tc.tile_pool(name="const", bufs=1))
        ident = cpool.tile([128, 128], F32)
        masks.make_identity(nc, ident[:])
        identb = cpool.tile([128, 128], BF16, name="identb")
        masks.make_identity(nc, identb[:])
        ones = cpool.tile([1, R], BF16, name="ones")
        nc.sync.dma_start(ones[:], ONES1.ap())
        einj = cpool.tile([16, 128], BF16, name="einj")
        nc.sync.dma_start(einj[:], EINJ.ap())
        e8sel = cpool.tile([128, 4 * CH], BF16, name="e8sel")
        nc.sync.dma_start(e8sel[:], E8SEL.ap())
        zr = cpool.tile([1, 128], BF16, name="zr")
        nc.gpsimd.memset(zr[:], 0.0)

        psp = ctx.enter_context(tc.tile_pool(name="ps", bufs=1, space="PSUM"))

        def gtile(idx, shape):
            return psp.tile(shape, F32, tag=f"g{idx}", name=f"g{idx}",
                            bufs=2 if idx < 2 else 1)

        # persistent SBUF xpart0: XPsb0[4*nb+b, t*512+c] = xpart0[b,t,512nb+c]
        xpp = ctx.enter_context(tc.tile_pool(name="xps", bufs=1))
        XPsb0 = xpp.tile([16, T * 512], BF16, name="xpsb0")
        # xpart1 double-banked chunk buffers: xc[par][nb][32b+tt, c]
        xc = [[xpp.tile([128, 512], BF16, name=f"xc{par}{nb}")
               for nb in range(4)] for par in range(2)]

        # transposed h2 history per layer: hT[p, j*256 + b*64 + t]
        hT = [xpp.tile([128, 4 * R], BF16, name=f"hT{l}") for l in range(L)]

        # ---------------- Phase A: xpart0 ----------------
        pa = ctx.enter_context(tc.tile_pool(name="pa", bufs=1))
        xt = [pa.tile([128, R], BF16, tag=f"xt{k}", name=f"xt{k}")
              for k in range(4)]
        xt.append(pa.tile([1, R], BF16, tag="xt4", name="xt4"))
        for k in range(4):
            nc.sync.dma_start(xt[k][:], XT0.ap()[128 * k:128 * (k + 1), :])
        nc.sync.dma_start(xt[4][:], XT0.ap()[512:513, :])
        wihA = [pa.tile([128, G], BF16, tag=f"wk{k}", name=f"wk{k}")
                for k in range(4)]
        wihA.append(pa.tile([1, G], BF16, tag="wk4", name="wk4"))
        for k in range(4):
            nc.sync.dma_start(wihA[k][:], WIH0.ap()[128 * k:128 * (k + 1), :])
        nc.sync.dma_start(wihA[4][:], WIH0.ap()[512:513, :])

        # prefetch pool: recurrence weights + attention operands
        pf = ctx.enter_context(tc.tile_pool(name="pf", bufs=1))
        whh_all = {}
        for l, Wd in ((0, WHH0), (1, WHH1)):
            tiles = [pf.tile([128, G], BF16, tag=f"whh{l}k{k}",
                             name=f"whh{l}k{k}") for k in range(4)]
            for k in range(4):
                nc.sync.dma_start(
                    tiles[k][:], Wd.ap()[128 * k:128 * (k + 1), :])
            whh_all[l] = tiles
        wint = [pf.tile([128, DS], BF16, tag=f"wi{k}", name=f"wi{k}")
                for k in range(4)]
        for k in range(4):
            nc.sync.dma_start(wint[k][:], WINT.ap()[128 * k:128 * (k + 1), :])
        encb = [pf.tile([S, DS], BF16, tag=f"enc{b}", name=f"enc{b}")
                for b in range(BS)]
        enctb = [[pf.tile([128, S], BF16, tag=f"ect{b}{k}", name=f"ect{b}{k}")
                  for k in range(8)] for b in range(BS)]
        offsb = [pf.tile([T, S], F32, tag=f"offs{b}", name=f"offs{b}")
                 for b in range(BS)]
        for b in range(BS):
            nc.sync.dma_start(encb[b][:], ENC.ap()[b])
            for k in range(8):
                nc.sync.dma_start(
                    enctb[b][k][:], ENCT.ap()[b, 128 * k:128 * (k + 1), :])
            nc.sync.dma_start(offsb[b][:], OFFS.ap()[b])

        def batched_xpart(wpool, lhs_tiles, wih_tiles):
            """Full-batch xpart0 = lhsT.T @ W -> XPsb0 (bf16, SBUF)."""
            for mc in range(2):
                for nb in range(4):
                    ps = gtile(nb % 2, [128, 512])
                    for k in range(5):
                        nc.tensor.matmul(
                            ps[:],
                            lhs_tiles[k][:, 128 * mc:128 * (mc + 1)],
                            wih_tiles[k][:, 512 * nb:512 * (nb + 1)],
                            start=(k == 0), stop=(k == 4))
                    sb = wpool.tile([128, 512], BF16, tag=f"stg{nb}",
                                    name=f"stg{nb}")
                    nc.scalar.copy(sb[:], ps[:])
                    dst = XPsb0[4 * nb + 2 * mc:4 * nb + 2 * mc + 2, :].rearrange(
                        "p (t c) -> p t c", c=512)
                    nc.sync.dma_start(dst, sb[:])

        batched_xpart(pa, xt, wihA)

        # reuse the wk tiles for W_ih1 (consumed by xpart1 chunks mid-pass)
        wihC = [pa.tile([128, G], BF16, tag=f"wk{k}", name=f"wc{k}")
                for k in range(4)]
        wihC.append(pa.tile([1, G], BF16, tag="wk4", name="wc4"))
        for k in range(4):
            nc.sync.dma_start(wihC[k][:], WIH1.ap()[128 * k:128 * (k + 1), :])
        nc.sync.dma_start(wihC[4][:], WIH1.ap()[512:513, :])

        # ---------------- Interleaved recurrence passes ----------------
        rp = ctx.enter_context(tc.tile_pool(name="rp", bufs=2))
        rc = ctx.enter_context(tc.tile_pool(name="rc", bufs=1))

        def linit(l):
            cT = rc.tile([128, 128], BF16, tag=f"cT{l}", name=f"cT{l}")
            nc.sync.dma_start(cT[:], CELLSP.ap()[l])
            c2T = rp.tile([128, 128], BF16, tag=f"c2T{l}", name=f"c2T{l}")
            nc.sync.dma_start(c2T[:], C2T0.ap()[l])
            return {"l": l, "cT": cT, "c2T": c2T, "whh": whh_all[l],
                    "h2_prev": None, "tprev": -1}

        def emit_h2_tail(st):
            tp2 = psp.tile([128, 128], BF16, tag="tp", name="tp", bufs=3)
            nc.tensor.transpose(tp2[:], st["h2_prev"][:], identb[:])
            src = tp2[:].rearrange("p (k r) -> p k r", k=4)[:, :, 0:BS]
            dst = hT[st["l"]][:].rearrange(
                "p (k b t) -> p k b t", k=4, b=BS)[:, :, :, st["tprev"]]
            nc.vector.tensor_copy(dst, src)

        def lstep(st, t):
            l = st["l"]
            gates = gtile(l, [128, 512])
            if l == 0:
                rhs = XPsb0[:, 512 * t:512 * (t + 1)]
                nc.tensor.matmul(gates[:], einj[:], rhs,
                                 start=True, stop=False, skip_group_check=True)
            else:
                tt4 = 4 * (t % CH)
                par = (t // CH) % 2
                nc.tensor.matmul(gates[:], zr[:], wihC[4][:, 0:512],
                                 start=True, stop=False, skip_group_check=True)
                for j in range(4):
                    nc.tensor.matmul(
                        gates[32 * j:32 * j + BS, :],
                        e8sel[:, tt4:tt4 + BS], xc[par][j][:],
                        start=False, stop=False,
                        tile_position=(0, 32 * j), skip_group_check=True)
            for k in range(4):
                lhsT = st["c2T"][:, 32 * k:32 * k + BS]
                for j in range(4):
                    nc.tensor.matmul(
                        gates[32 * j:32 * j + BS, :],
                        lhsT, st["whh"][k][:, 512 * j:512 * (j + 1)],
                        start=False, stop=(k == 3),
                        tile_position=(0, 32 * j), skip_group_check=True)
            sif = rp.tile([128, 256], BF16, tag=f"sif{l}", name=f"sif{l}")
            nc.scalar.activation(sif[:], gates[:, 0:256], AF.Sigmoid)
            tg = rp.tile([128, 128], BF16, tag=f"tg{l}", name=f"tg{l}")
            nc.scalar.activation(tg[:], gates[:, 384:512], AF.Tanh)
            m1 = rp.tile([128, 128], BF16, tag=f"m1{l}", name=f"m1{l}")
            nc.vector.tensor_mul(m1[:], sif[:, 128:256], st["cT"][:])
            m2 = rp.tile([128, 128], BF16, tag=f"m2{l}", name=f"m2{l}")
            nc.vector.tensor_mul(m2[:], sif[:, 0:128], tg[:])
            c2h = rp.tile([128, 128], BF16, tag=f"c2h{l}", name=f"c2h{l}")
            nc.vector.tensor_add(c2h[:], m1[:], m2[:])
            tp = psp.tile([128, 128], BF16, tag="tp", name="tp", bufs=3)
            nc.tensor.transpose(tp[:], c2h[:], identb[:])
            c2T_new = rp.tile([128, 128], BF16, tag=f"c2T{l}", name=f"c2T{l}")
            nc.vector.tensor_copy(c2T_new[:], tp[:])
            # h2 branch (off the c2 critical chain)
            so = rp.tile([128, 128], BF16, tag=f"so{l}", name=f"so{l}")
            nc.scalar.activation(so[:], gates[:, 256:384], AF.Sigmoid)
            tc2 = rp.tile([128, 128], BF16, tag=f"tc2{l}", name=f"tc2{l}")
            nc.scalar.activation(tc2[:], c2h[:], AF.Tanh)
            h2 = rp.tile([128, 128], BF16, tag=f"h2{l}", name=f"h2{l}")
            nc.vector.tensor_mul(h2[:], so[:], tc2[:])
            if st["h2_prev"] is not None:
                emit_h2_tail(st)
            st["h2_prev"] = h2
            st["tprev"] = t
            st["c2T"] = c2T_new

        def emit_chunk(c):
            """xpart1 for steps [8c, 8c+8) -> xc[c%2] chunk buffers."""
            base = CH * c
            par = c % 2
            for nb in range(4):
                nsl = slice(512 * nb, 512 * (nb + 1))
                ps = psp.tile([128, 512], F32, tag="g2", name="g2", bufs=1)
                nc.tensor.matmul(ps[:], ones[0:1, 0:128], wihC[4][:, nsl],
                                 start=True, stop=False, skip_group_check=True)
                for k in range(4):
                    for b in range(BS):
                        lhsT = hT[0][:, 256 * k + 64 * b + base:
                                     256 * k + 64 * b + base + CH]
                        nc.tensor.matmul(
                            ps[32 * b:32 * b + CH, :], lhsT, wihC[k][:, nsl],
                            start=False, stop=(k == 3),
                            tile_position=(0, 32 * b), skip_group_check=True)
                nc.vector.tensor_copy(xc[par][nb][:], ps[:])

        st0 = linit(0)
        st1 = linit(1)
        for ss in range(T + LAG + 1):
            if ss < T:
                lstep(st0, ss)
            if ss == T:
                emit_h2_tail(st0)            # flush T_h2(0, 63)
                emit_chunk(7)
            if 9 <= ss < T and (ss - 9) % CH == 0:
                emit_chunk((ss - 9) // CH)
            if LAG <= ss < T + LAG:
                lstep(st1, ss - LAG)
            if ss == T + LAG:
                emit_h2_tail(st1)            # flush T_h2(1, 63)

        # ---------------- Phase E: attention + out proj ----------------
        # reuse wk tiles once more for W_out (packed 4 chunks per tile)
        wkt = [pa.tile([128, G], BF16, tag=f"wk{k}", name=f"wot{k}")
               for k in range(3)]
        for k in range(12):
            nc.sync.dma_start(
                wkt[k // 4][:, 512 * (k % 4):512 * (k % 4 + 1)],
                WOUTT.ap()[128 * k:128 * (k + 1), :])
        woutb = pa.tile([1, G], BF16, tag="wk4", name="wob")
        nc.sync.dma_start(woutb[:, 0:512], WOUTT.ap()[1536:1537, :])

        with tc.tile_pool(name="pe", bufs=1) as pe:
            sT = [hT[1][:, 256 * k:256 * (k + 1)] for k in range(4)]

            xqT = []
            for m in range(8):
                ps = gtile(m % 2, [128, R])
                for k in range(4):
                    nc.tensor.matmul(
                        ps[:], wint[k][:, 128 * m:128 * (m + 1)], sT[k],
                        start=(k == 0), stop=(k == 3))
                xq = pe.tile([128, R], BF16, tag=f"xq{m}", name=f"xq{m}")
                if m % 2 == 0:
                    nc.scalar.copy(xq[:], ps[:])
                else:
                    nc.vector.tensor_copy(xq[:], ps[:])
                xqT.append(xq)

            ctxT = [pe.tile([128, R], BF16, tag=f"cx{m}", name=f"cx{m}")
                    for m in range(8)]
            for b in range(BS):
                bsl = slice(T * b, T * (b + 1))
                eps = psp.tile([T, S], F32, tag="g2", name="g2", bufs=1)
                for k in range(8):
                    nc.tensor.matmul(
                        eps[:], xqT[k][:, bsl], enctb[b][k][:],
                        start=(k == 0), stop=(k == 7))
                esb = pe.tile([T, S], F32, tag=f"esb{b % 2}",
                              name=f"esb{b % 2}")
                nc.vector.tensor_add(esb[:], eps[:], offsb[b][:])
                negmax = pe.tile([T, 1], F32, tag=f"negmax{b % 2}",
                                 name=f"negmax{b % 2}")
                nc.vector.reduce_max(
                    negmax[:], esb[:], axis=mybir.AxisListType.X, negate=True)
                expE = pe.tile([T, S], F32, tag=f"expE{b % 2}",
                               name=f"expE{b % 2}")
                den = pe.tile([T, 1], F32, tag=f"den{b % 2}",
                              name=f"den{b % 2}")
                nc.scalar.activation(
                    expE[:], esb[:], AF.Exp, bias=negmax[:], accum_out=den[:])
                rden = pe.tile([T, 1], F32, tag=f"rden{b % 2}",
                               name=f"rden{b % 2}")
                nc.vector.reciprocal(rden[:], den[:])
                attn = pe.tile([T, S], F32, tag=f"attn{b % 2}",
                               name=f"attn{b % 2}")
                nc.vector.tensor_scalar_mul(attn[:], expE[:], rden[:])
                tpa = psp.tile([S, T], F32, tag="tp", name="tp", bufs=3)
                nc.tensor.transpose(tpa[:], attn[:], ident[0:T, 0:T])
                atsb = pe.tile([S, T], BF16, tag=f"atsb{b % 2}",
                               name=f"atsb{b % 2}")
                nc.vector.tensor_copy(atsb[:], tpa[:])
                for m in range(8):
                    psc = gtile(m % 2, [128, T])
                    nc.tensor.matmul(
                        psc[:], encb[b][:, 128 * m:128 * (m + 1)], atsb[:],
                        start=True, stop=True)
                    if m % 2 == 0:
                        nc.scalar.copy(ctxT[m][:, bsl], psc[:])
                    else:
                        nc.vector.tensor_copy(ctxT[m][:, bsl], psc[:])

            outflat = OUT.ap().rearrange("b t d -> (b t) d")
            lhs_all = ctxT + sT + [ones]
            wt_all = [wkt[k // 4][:, 512 * (k % 4):512 * (k % 4 + 1)]
                      for k in range(12)] + [woutb[:, 0:512]]
            for mc in range(2):
                msl = slice(128 * mc, 128 * (mc + 1))
                ps = gtile(mc, [128, D])
                for k in range(13):
                    nc.tensor.matmul(
                        ps[:], lhs_all[k][:, msl], wt_all[k],
                        start=(k == 0), stop=(k == 12))
                osb = pe.tile([128, D], F32, tag=f"osb{mc}", name=f"osb{mc}")
                nc.scalar.activation(osb[:], ps[:], AF.Tanh)
                nc.sync.dma_start(outflat[msl, :], osb[:])

    nc.compile()
    return nc


def assemble(results):
    full = np.concatenate([r["out"] for r in results], axis=0)  # [B, T, D]
    outs = full.transpose(1, 0, 2)                              # [T, B, D]
    return np.ascontiguousarray(outs.reshape(-1, D).reshape(-1, T, D))


_nc_cache = None


def kernel(**inputs):
    global _nc_cache
    in_maps = host_prep(inputs)
    if _nc_cache is None:
        _nc_cache = build_program()
    res = run_bass_kernel_spmd(_nc_cache, in_maps, list(range(NCORES)))
    return assemble(res.results)


# revision 17
# speedup vs baseline: 1.7717x; 1.3876x over previous
"""Trainium2 Bass kernel for nn_Decoder_46042049413334.

Buggy 2-layer LSTM decoder with attention (B=32, T=64, S=128, D=512).

Structure (per core, batch sharded 8 ways, BS=4 examples/core):
  Phase A: xpart0 = [emb(tokens), 1] @ [W_ih0.T; b0]  -> XPsb0 (SBUF)
  Interleaved pass: layer-0 step t and layer-1 step t-2 run together;
    layer-1's xpart is accumulated per step directly into its gates
    PSUM from the transposed h2 history (hT0), so the PE stays busy
    enough to hold the HAM clock gate open (K=8/8).
  Phase E: attention + out-projection from hT1

Recurrence layout: gates PSUM [128, 512] where partition 32*j+b holds
(example b, d-block j) and the 512 free cols are {i,f,o,2g}x128 for
that d-block (g columns pre-scaled by 2 so tanh(g) = 2*sigmoid(2g)-1
comes out of a single full-width sigmoid). The four d-blocks' weight
streams run CONCURRENTLY in the PE array via tile_position=(0, 32*j)
column tiling. Elementwise runs once over all 128 partitions; c2 and
h2 are re-transposed per step ([128,128] PE transpose). Off-chain work
(tanh(c2), h2, its transpose/gather) is emitted one step late so no
engine FIFO ever blocks the recurrence chains.

Row ordering is b-major everywhere: row r = b_local*T + t.
"""
import numpy as np
import ml_dtypes
from contextlib import ExitStack

import concourse.bass as bass
import concourse.bacc as bacc
import concourse.tile as tile
from concourse import mybir, masks
from concourse.bass_utils import run_bass_kernel_spmd

F32 = mybir.dt.float32
BF16 = mybir.dt.bfloat16
AF = mybir.ActivationFunctionType
NPBF = ml_dtypes.bfloat16

B, T, S, D, L, V = 32, 64, 128, 512, 2, 32000
G = 4 * D        # 2048
DS = 2 * D       # 1024
NCORES = 8
BS = B // NCORES  # 4
R = BS * T        # 256 rows per core
LAG = 2          # layer-1 recurrence lag behind layer 0


# ---------------------------------------------------------------- host side

def _gate_perm():
    perm = np.zeros(G, dtype=np.int64)
    base = {0: 0, 1: 512, 2: 1536, 3: 1024}  # i, f, o, g
    for j in range(G):
        nb, pos = divmod(j, 512)
        sub, dd = divmod(pos, 128)
        perm[j] = base[sub] + nb * 128 + dd
    return perm


def host_prep(inputs):
    """Build the 8 per-core input maps (layout/gather work only)."""
    perm = _gate_perm()
    tokens = np.asarray(inputs["prev_tgt_tokens"])
    embed = np.asarray(inputs["embed"], dtype=np.float32)
    enc = np.asarray(inputs["encoder_out"], dtype=np.float32)
    mask = np.asarray(inputs["src_mask"])
    hid = np.asarray(inputs["hiddens"], dtype=np.float32)
    cells = np.asarray(inputs["cells"], dtype=np.float32)
    W_ih = np.asarray(inputs["W_ih"], dtype=np.float32)
    W_hh = np.asarray(inputs["W_hh"], dtype=np.float32)
    b_ih = np.asarray(inputs["b_ih"], dtype=np.float32)
    b_hh = np.asarray(inputs["b_hh"], dtype=np.float32)
    W_in = np.asarray(inputs["W_in"], dtype=np.float32)
    b_in = np.asarray(inputs["b_in"], dtype=np.float32)
    W_out = np.asarray(inputs["W_out"], dtype=np.float32)
    b_out = np.asarray(inputs["b_out"], dtype=np.float32)

    def bf(x):
        return np.ascontiguousarray(x, dtype=NPBF)

    WIH = []
    WHH = []
    gscale = np.ones(G, np.float32)
    for nb in range(4):
        gscale[512 * nb + 384:512 * (nb + 1)] = 2.0   # tanh(g)=2*sig(2g)-1
    for l in range(L):
        wihT = W_ih[l].T[:, perm] * gscale
        biasrow = ((b_ih[l] + b_hh[l])[perm] * gscale)[None, :]
        WIH.append(bf(np.concatenate([wihT, biasrow], 0)))   # [513, 2048]
        WHH.append(bf(W_hh[l].T[:, perm] * gscale))          # [512, 2048]
    WINT = bf(W_in.T)                                        # [512, 1024]
    WOUTT = bf(np.concatenate([W_out.T, b_out[None, :]], 0))  # [1537, 512]

    # K=16 xpart injection selector: E[4j+b, 32j+b] = 1
    einj = np.zeros((16, 128), np.float32)
    for j in range(4):
        for b in range(BS):
            einj[4 * j + b, 32 * j + b] = 1.0
    einj = bf(einj)

    # block selector: e4blk[j, p] = 1 iff p // 32 == j
    e4 = np.zeros((4, 128), np.float32)
    for j in range(4):
        e4[j, 32 * j:32 * (j + 1)] = 1.0
    e4 = bf(e4)

    in_maps = []
    for core in range(NCORES):
        bsl = slice(core * BS, (core + 1) * BS)
        xe = embed[tokens[bsl]]                              # [BS, T, D]
        Xaug = np.concatenate(
            [xe.reshape(R, D), np.ones((R, 1), np.float32)], axis=1)
        XT0 = bf(Xaug.T)                                     # [513, 256]
        enc_c = np.ascontiguousarray(enc[bsl])               # [BS, 128, 1024]
        encT_c = np.swapaxes(enc_c, 1, 2)                    # [BS, 1024, 128]
        offs = np.einsum("bsd,d->bs", enc_c, b_in) + np.where(mask[bsl], -1e9, 0.0)
        offs_rep = np.ascontiguousarray(
            np.broadcast_to(offs[:, None, :], (BS, T, S)), dtype=np.float32)
        # initial c2T: c2t0[l, p, 32k+b] = hid[l, b, 128k+p]
        th = hid[:, bsl].reshape(L, BS, 4, 128).transpose(0, 3, 2, 1)  # [L,128,4,BS]
        c2t0 = np.zeros((L, 128, 4, 32), np.float32)
        c2t0[:, :, :, 0:BS] = th
        c2t0 = bf(c2t0.reshape(L, 128, 128))
        # cells in partition layout: cellsp[l, 32j+b, p] = cells[l, b, 128j+p]
        tc_ = cells[:, bsl].reshape(L, BS, 4, 128).transpose(0, 2, 1, 3)  # [L,4,BS,128]
        cellsp = np.zeros((L, 4, 32, 128), np.float32)
        cellsp[:, :, 0:BS, :] = tc_
        cellsp = bf(cellsp.reshape(L, 128, 128))
        in_maps.append({
            "xt0": XT0,
            "wih0": WIH[0], "whh0": WHH[0],
            "wih1": WIH[1], "whh1": WHH[1],
            "wint": WINT, "woutt": WOUTT,
            "enc": bf(enc_c), "enct": bf(encT_c), "offs": offs_rep,
            "c2t0": c2t0, "cellsp": cellsp,
            "ones1": np.ones((1, R), NPBF),
            "einj": einj, "e4blk": e4,
        })
    return in_maps


# ------------------------------------------------------------- device build

def build_program():
    nc = bacc.Bacc("TRN2", target_bir_lowering=False, debug=False)

    XT0 = nc.dram_tensor("xt0", [513, R], BF16, kind="ExternalInput")
    WIH0 = nc.dram_tensor("wih0", [513, G], BF16, kind="ExternalInput")
    WHH0 = nc.dram_tensor("whh0", [D, G], BF16, kind="ExternalInput")
    WIH1 = nc.dram_tensor("wih1", [513, G], BF16, kind="ExternalInput")
    WHH1 = nc.dram_tensor("whh1", [D, G], BF16, kind="ExternalInput")
    WINT = nc.dram_tensor("wint", [D, DS], BF16, kind="ExternalInput")
    WOUTT = nc.dram_tensor("woutt", [DS + D + 1, D], BF16, kind="ExternalInput")
    ENC = nc.dram_tensor("enc", [BS, S, DS], BF16, kind="ExternalInput")
    ENCT = nc.dram_tensor("enct", [BS, DS, S], BF16, kind="ExternalInput")
    OFFS = nc.dram_tensor("offs", [BS, T, S], F32, kind="ExternalInput")
    C2T0 = nc.dram_tensor("c2t0", [L, 128, 128], BF16, kind="ExternalInput")
    CELLSP = nc.dram_tensor("cellsp", [L, 128, 128], BF16, kind="ExternalInput")
    ONES1 = nc.dram_tensor("ones1", [1, R], BF16, kind="ExternalInput")
    EINJ = nc.dram_tensor("einj", [16, 128], BF16, kind="ExternalInput")
    E4BLK = nc.dram_tensor("e4blk", [4, 128], BF16, kind="ExternalInput")
    OUT = nc.dram_tensor("out", [BS, T, D], F32, kind="ExternalOutput")

    with tile.TileContext(nc) as tc, ExitStack() as ctx:
        cpool = ctx.enter_context(tc.tile_pool(name="const", bufs=1))
        ident = cpool.tile([128, 128], F32)
        masks.make_identity(nc, ident[:])
        identb = cpool.tile([128, 128], BF16, name="identb")
        masks.make_identity(nc, identb[:])
        ones = cpool.tile([1, R], BF16, name="ones")
        nc.sync.dma_start(ones[:], ONES1.ap())
        einj = cpool.tile([16, 128], BF16, name="einj")
        nc.sync.dma_start(einj[:], EINJ.ap())
        e4blk = cpool.tile([4, 128], BF16, name="e4blk")
        nc.sync.dma_start(e4blk[:], E4BLK.ap())
        ones128 = cpool.tile([128, 128], BF16, name="ones128")
        nc.gpsimd.memset(ones128[:], 1.0)

        psp = ctx.enter_context(tc.tile_pool(name="ps", bufs=1, space="PSUM"))

        def gtile(idx, shape):
            return psp.tile(shape, F32, tag=f"g{idx}", name=f"g{idx}",
                            bufs=2 if idx < 2 else 1)

        # persistent SBUF xpart0: XPsb0[4*nb+b, t*512+c] = xpart0[b,t,512nb+c]
        xpp = ctx.enter_context(tc.tile_pool(name="xps", bufs=1))
        XPsb0 = xpp.tile([16, T * 512], BF16, name="xpsb0")

        # transposed h2 history per layer: hT[p, k*256 + b*64 + t]
        hT = [xpp.tile([128, 4 * R], BF16, name=f"hT{l}") for l in range(L)]

        # ---------------- Phase A inputs (packed, few DMAs) ----------------
        pa = ctx.enter_context(tc.tile_pool(name="pa", bufs=1))
        xtt = pa.tile([128, 4 * R], BF16, tag="xtt", name="xtt")
        nc.sync.dma_start(
            xtt[:].rearrange("p (k c) -> p k c", k=4),
            XT0.ap()[0:512].rearrange("(k p) c -> p k c", k=4))
        xt4 = pa.tile([1, R], BF16, tag="xt4", name="xt4")
        nc.sync.dma_start(xt4[:], XT0.ap()[512:513, :])
        wkt = pa.tile([128, 4 * G], BF16, tag="wkt", name="wkt")
        nc.sync.dma_start(
            wkt[:].rearrange("p (k c) -> p k c", k=4),
            WIH0.ap()[0:512].rearrange("(k p) c -> p k c", k=4))
        wk4 = pa.tile([1, G], BF16, tag="wk4", name="wk4")
        nc.sync.dma_start(wk4[:], WIH0.ap()[512:513, :])

        # PE warm-up: dummy matmuls on the identity while DMAs land
        wps = psp.tile([128, 128], F32, tag="g2", name="g2", bufs=1)
        for w in range(48):
            nc.tensor.matmul(wps[:], identb[:], identb[:],
                             start=True, stop=True, skip_group_check=True)

        # prefetch pool: recurrence weights + attention operands
        pf = ctx.enter_context(tc.tile_pool(name="pf", bufs=1))
        whht = []
        for l, Wd in ((0, WHH0), (1, WHH1)):
            wt = pf.tile([128, 4 * G], BF16, tag=f"whh{l}", name=f"whh{l}")
            nc.sync.dma_start(
                wt[:].rearrange("p (k c) -> p k c", k=4),
                Wd.ap().rearrange("(k p) c -> p k c", k=4))
            whht.append(wt)
        wihC = pf.tile([128, 4 * G], BF16, tag="wihC", name="wihC")
        nc.sync.dma_start(
            wihC[:].rearrange("p (k c) -> p k c", k=4),
            WIH1.ap()[0:512].rearrange("(k p) c -> p k c", k=4))
        wihC4 = pf.tile([4, 512], BF16, tag="wihC4", name="wihC4")
        nc.sync.dma_start(wihC4[:], WIH1.ap()[512:513, :].rearrange(
            "a (j c) -> (a j) c", j=4))
        wint = pf.tile([128, 4 * DS], BF16, tag="wint", name="wint")
        nc.sync.dma_start(
            wint[:].rearrange("p (k c) -> p k c", k=4),
            WINT.ap().rearrange("(k p) c -> p k c", k=4))
        cTb = pf.tile([128, 2 * 128], BF16, tag="cTb", name="cTb")
        nc.sync.dma_start(
            cTb[:].rearrange("p (l c) -> p l c", l=2),
            CELLSP.ap().rearrange("l p c -> p l c"))
        c2T0b = pf.tile([128, 2 * 128], BF16, tag="c2T0b", name="c2T0b")
        nc.sync.dma_start(
            c2T0b[:].rearrange("p (l c) -> p l c", l=2),
            C2T0.ap().rearrange("l p c -> p l c"))
        encb = pf.tile([S, 4 * DS], BF16, tag="encb", name="encb")
        nc.sync.dma_start(
            encb[:].rearrange("p (b c) -> p b c", b=BS),
            ENC.ap().rearrange("b s d -> s b d"))
        enctb = pf.tile([128, BS * 8 * S], BF16, tag="enctb", name="enctb")
        nc.sync.dma_start(
            enctb[:].rearrange("p (b k s) -> p b k s", b=BS, k=8),
            ENCT.ap().rearrange("b (k p) s -> p b k s", k=8))
        offsb = pf.tile([T, BS * S], F32, tag="offsb", name="offsb")
        nc.sync.dma_start(
            offsb[:].rearrange("p (b s) -> p b s", b=BS),
            OFFS.ap().rearrange("b t s -> t b s"))
        woutt = pf.tile([128, 12 * D], BF16, tag="woutt", name="woutt")
        nc.sync.dma_start(
            woutt[:].rearrange("p (k c) -> p k c", k=12),
            WOUTT.ap()[0:1536].rearrange("(k p) c -> p k c", k=12))
        woutb = pf.tile([1, D], BF16, tag="woutb", name="woutb")
        nc.sync.dma_start(woutb[:], WOUTT.ap()[1536:1537, :])

        # ---------------- Phase A: xpart0 ----------------
        for mc in range(2):
            for nb in range(4):
                ps = gtile(nb % 2, [128, 512])
                for k in range(4):
                    nc.tensor.matmul(
                        ps[:],
                        xtt[:, 256 * k + 128 * mc:256 * k + 128 * (mc + 1)],
                        wkt[:, 2048 * k + 512 * nb:2048 * k + 512 * (nb + 1)],
                        start=(k == 0), stop=False)
                nc.tensor.matmul(
                    ps[:], xt4[:, 128 * mc:128 * (mc + 1)],
                    wk4[:, 512 * nb:512 * (nb + 1)],
                    start=False, stop=True)
                sb = pa.tile([128, 512], BF16, tag=f"stg{nb}", name=f"stg{nb}")
                nc.scalar.copy(sb[:], ps[:])
                dst = XPsb0[4 * nb + 2 * mc:4 * nb + 2 * mc + 2, :].rearrange(
                    "p (t c) -> p t c", c=512)
                nc.sync.dma_start(dst, sb[:])

        # ---------------- Interleaved recurrence passes ----------------
        rp = ctx.enter_context(tc.tile_pool(name="rp", bufs=2))

        def linit(l):
            return {"l": l, "cT": cTb[:, 128 * l:128 * (l + 1)],
                    "c2T": c2T0b[:, 128 * l:128 * (l + 1)], "whh": whht[l],
                    "sall_prev": None, "c2h_prev": None, "tprev": -1}

        def lstep_mm(st, t):
            """Inject/xpart + W_hh rounds for step t (PE bulk)."""
            l = st["l"]
            gates = gtile(l, [128, 512])
            if l == 0:
                rhs = XPsb0[:, 512 * t:512 * (t + 1)]
                nc.tensor.matmul(gates[:], einj[:], rhs,
                                 start=True, stop=False, skip_group_check=True)
            else:
                # xpart1(t) accumulated in place: bias row first (writes all
                # 128 partitions), then W_ih1 rounds from hT0 columns of t
                nc.tensor.matmul(gates[:], e4blk[:], wihC4[:],
                                 start=True, stop=False, skip_group_check=True)
                for k in range(4):
                    lhsT = hT[0][:].rearrange(
                        "p (k b t) -> p k b t", k=4, b=BS)[:, k, :, t]
                    for j in range(4):
                        nc.tensor.matmul(
                            gates[32 * j:32 * j + BS, :],
                            lhsT,
                            wihC[:, 2048 * k + 512 * j:2048 * k + 512 * (j + 1)],
                            start=False, stop=False,
                            tile_position=(0, 32 * j), skip_group_check=True)
            for k in range(4):
                lhsT = st["c2T"][:, 32 * k:32 * k + BS]
                for j in range(4):
                    nc.tensor.matmul(
                        gates[32 * j:32 * j + BS, :],
                        lhsT,
                        st["whh"][:, 2048 * k + 512 * j:2048 * k + 512 * (j + 1)],
                        start=False, stop=(k == 3),
                        tile_position=(0, 32 * j), skip_group_check=True)
            st["gates"] = gates

        def lstep_tail(st, t):
            """sigma + c2 chain + transpose + cast for step t."""
            l = st["l"]
            gates = st["gates"]
            sall = rp.tile([128, 512], F32, tag=f"sa{l}", name=f"sa{l}")
            nc.scalar.activation(sall[:], gates[:], AF.Sigmoid)
            m1 = rp.tile([128, 128], BF16, tag=f"m1{l}", name=f"m1{l}")
            nc.vector.tensor_mul(m1[:], sall[:, 128:256], st["cT"])
            tgv = rp.tile([128, 128], BF16, tag=f"tg{l}", name=f"tg{l}")
            nc.vector.scalar_tensor_tensor(
                tgv[:], sall[:, 384:512], 2.0, ones128[:],
                mybir.AluOpType.mult, mybir.AluOpType.subtract)
            m2 = rp.tile([128, 128], BF16, tag=f"m2{l}", name=f"m2{l}")
            nc.vector.tensor_mul(m2[:], sall[:, 0:128], tgv[:])
            c2h = rp.tile([128, 128], BF16, tag=f"c2h{l}", name=f"c2h{l}")
            nc.vector.tensor_add(c2h[:], m1[:], m2[:])
            tp = psp.tile([128, 128], BF16, tag="tp", name="tp", bufs=3)
            nc.tensor.transpose(tp[:], c2h[:], identb[:])
            c2T_new = rp.tile([128, 128], BF16, tag=f"c2T{l}", name=f"c2T{l}")
            nc.vector.tensor_copy(c2T_new[:], tp[:])
            st["sall"] = sall
            st["c2h"] = c2h
            st["c2T"] = c2T_new

        def lstep_branch(st):
            """Delayed h2 branch for the PREVIOUS step (never blocks chains)."""
            l = st["l"]
            if st["sall_prev"] is not None:
                tc2 = rp.tile([128, 128], BF16, tag=f"tc2{l}", name=f"tc2{l}")
                nc.scalar.activation(tc2[:], st["c2h_prev"][:], AF.Tanh)
                h2 = rp.tile([128, 128], BF16, tag=f"h2{l}", name=f"h2{l}")
                nc.gpsimd.tensor_mul(h2[:], st["sall_prev"][:, 256:384], tc2[:])
                tp2 = psp.tile([128, 128], BF16, tag="tp", name="tp", bufs=3)
                nc.tensor.transpose(tp2[:], h2[:], identb[:])
                src_ = tp2[:].rearrange("p (k r) -> p k r", k=4)[:, :, 0:BS]
                dst = hT[l][:].rearrange(
                    "p (k b t) -> p k b t", k=4, b=BS)[:, :, :, st["tprev"]]
                nc.vector.tensor_copy(dst, src_)
            st["sall_prev"] = st["sall"]
            st["c2h_prev"] = st["c2h"]
            st["tprev"] = st["tprev"] + 1

        st0 = linit(0)
        st1 = linit(1)
        for ss in range(T + LAG + 1):
            if ss < T:
                lstep_mm(st0, ss)
            if LAG <= ss < T + LAG:
                lstep_mm(st1, ss - LAG)
            if ss < T:
                lstep_tail(st0, ss)
            if LAG <= ss < T + LAG:
                lstep_tail(st1, ss - LAG)
            if ss <= T:
                lstep_branch(st0)
            if LAG <= ss <= T + LAG:
                lstep_branch(st1)

        # ---------------- Phase E: attention + out proj ----------------
        wkt2 = pa.tile([128, 4 * G], BF16, tag="wkt", name="wkt2")
        with tc.tile_pool(name="pe", bufs=1) as pe:
            sT = [hT[1][:, 256 * k:256 * (k + 1)] for k in range(4)]

            xqT = []
            for m in range(8):
                ps = gtile(m % 2, [128, R])
                for k in range(4):
                    nc.tensor.matmul(
                        ps[:], wint[:, 1024 * k + 128 * m:1024 * k + 128 * (m + 1)],
                        sT[k], start=(k == 0), stop=(k == 3))
                xq = wkt2[:, 256 * m:256 * (m + 1)]
                if m % 2 == 0:
                    nc.scalar.copy(xq, ps[:])
                else:
                    nc.vector.tensor_copy(xq, ps[:])
                xqT.append(xq)

            ctxT = [wkt2[:, 2048 + 256 * m:2048 + 256 * (m + 1)]
                    for m in range(8)]
            for b in range(BS):
                bsl = slice(T * b, T * (b + 1))
                eps = psp.tile([T, S], F32, tag="g2", name="g2", bufs=1)
                for k in range(8):
                    nc.tensor.matmul(
                        eps[:], xqT[k][:, bsl],
                        enctb[:, 1024 * b + 128 * k:1024 * b + 128 * (k + 1)],
                        start=(k == 0), stop=(k == 7))
                esb = pe.tile([T, S], F32, tag=f"esb{b % 2}",
                              name=f"esb{b % 2}")
                nc.vector.tensor_add(esb[:], eps[:],
                                     offsb[:, 128 * b:128 * (b + 1)])
                negmax = pe.tile([T, 1], F32, tag=f"negmax{b % 2}",
                                 name=f"negmax{b % 2}")
                nc.vector.reduce_max(
                    negmax[:], esb[:], axis=mybir.AxisListType.X, negate=True)
                expE = pe.tile([T, S], F32, tag=f"expE{b % 2}",
                               name=f"expE{b % 2}")
                den = pe.tile([T, 1], F32, tag=f"den{b % 2}",
                              name=f"den{b % 2}")
                nc.scalar.activation(
                    expE[:], esb[:], AF.Exp, bias=negmax[:], accum_out=den[:])
                rden = pe.tile([T, 1], F32, tag=f"rden{b % 2}",
                               name=f"rden{b % 2}")
                nc.vector.reciprocal(rden[:], den[:])
                attn = pe.tile([T, S], F32, tag=f"attn{b % 2}",
                               name=f"attn{b % 2}")
                nc.vector.tensor_scalar_mul(attn[:], expE[:], rden[:])
                tpa = psp.tile([S, T], F32, tag="tp", name="tp", bufs=3)
                nc.tensor.transpose(tpa[:], attn[:], ident[0:T, 0:T])
                atsb = pe.tile([S, T], BF16, tag=f"atsb{b % 2}",
                               name=f"atsb{b % 2}")
                nc.vector.tensor_copy(atsb[:], tpa[:])
                for m in range(8):
                    psc = gtile(m % 2, [128, T])
                    nc.tensor.matmul(
                        psc[:], encb[:, 1024 * b + 128 * m:1024 * b + 128 * (m + 1)],
                        atsb[:], start=True, stop=True)
                    if m % 2 == 0:
                        nc.scalar.copy(ctxT[m][:, bsl], psc[:])
                    else:
                        nc.vector.tensor_copy(ctxT[m][:, bsl], psc[:])

            outflat = OUT.ap().rearrange("b t d -> (b t) d")
            lhs_all = ctxT + sT + [ones]
            wt_all = [woutt[:, 512 * k:512 * (k + 1)] for k in range(12)] \
                + [woutb[:]]
            for mc in range(2):
                msl = slice(128 * mc, 128 * (mc + 1))
                ps = gtile(mc, [128, D])
                for k in range(13):
                    nc.tensor.matmul(
                        ps[:], lhs_all[k][:, msl], wt_all[k],
                        start=(k == 0), stop=(k == 12))
                osb = pa.tile([128, D], F32, tag=f"stg{mc}", name=f"osb{mc}")
                nc.scalar.activation(osb[:], ps[:], AF.Tanh)
                nc.sync.dma_start(outflat[msl, :], osb[:])

    nc.compile()
    return nc


def assemble(results):
    full = np.concatenate([r["out"] for r in results], axis=0)  # [B, T, D]
    outs = full.transpose(1, 0, 2)                              # [T, B, D]
    return np.ascontiguousarray(outs.reshape(-1, D).reshape(-1, T, D))


_nc_cache = None


def kernel(**inputs):
    global _nc_cache
    in_maps = host_prep(inputs)
    if _nc_cache is None:
        _nc_cache = build_program()
    res = run_bass_kernel_spmd(_nc_cache, in_maps, list(range(NCORES)))
    return assemble(res.results)


# revision 19
# speedup vs baseline: 1.9282x; 1.0884x over previous
"""Trainium2 Bass kernel for nn_Decoder_46042049413334.

Buggy 2-layer LSTM decoder with attention (B=32, T=64, S=128, D=512).

Structure (per core, batch sharded 8 ways, BS=4 examples/core):
  Phase A: xpart0 = [emb(tokens), 1] @ [W_ih0.T; b0]  -> XPsb0 (SBUF)
  Interleaved pass: layer-0 step t and layer-1 step t-2 run together;
    layer-1's xpart is accumulated per step directly into its gates
    PSUM from the transposed h2 history (hT0), so the PE stays busy
    enough to hold the HAM clock gate open (K=8/8).
  Phase E: attention + out-projection from hT1

Recurrence layout: gates PSUM [128, 512] where partition 32*j+b holds
(example b, d-block j) and the 512 free cols are {i,f,o,2g}x128 for
that d-block (g columns pre-scaled by 2 so tanh(g) = 2*sigmoid(2g)-1
comes out of a single full-width sigmoid). The four d-blocks' weight
streams run CONCURRENTLY in the PE array via tile_position=(0, 32*j)
column tiling. Elementwise runs once over all 128 partitions; c2 and
h2 are re-transposed per step ([128,128] PE transpose). Off-chain work
(tanh(c2), h2, its transpose/gather) is emitted one step late so no
engine FIFO ever blocks the recurrence chains.

Row ordering is b-major everywhere: row r = b_local*T + t.
"""
import numpy as np
import ml_dtypes
from contextlib import ExitStack

import concourse.bass as bass
import concourse.bacc as bacc
import concourse.tile as tile
from concourse import mybir, masks
from concourse.bass_utils import run_bass_kernel_spmd

F32 = mybir.dt.float32
BF16 = mybir.dt.bfloat16
AF = mybir.ActivationFunctionType
NPBF = ml_dtypes.bfloat16

B, T, S, D, L, V = 32, 64, 128, 512, 2, 32000
G = 4 * D        # 2048
DS = 2 * D       # 1024
NCORES = 8
BS = B // NCORES  # 4
R = BS * T        # 256 rows per core
LAG = 2          # layer-1 recurrence lag behind layer 0


# ---------------------------------------------------------------- host side

def _gate_perm():
    perm = np.zeros(G, dtype=np.int64)
    base = {0: 0, 1: 512, 2: 1536, 3: 1024}  # i, f, o, g
    for j in range(G):
        nb, pos = divmod(j, 512)
        sub, dd = divmod(pos, 128)
        perm[j] = base[sub] + nb * 128 + dd
    return perm


def host_prep(inputs):
    """Build the 8 per-core input maps (layout/gather work only)."""
    perm = _gate_perm()
    tokens = np.asarray(inputs["prev_tgt_tokens"])
    embed = np.asarray(inputs["embed"], dtype=np.float32)
    enc = np.asarray(inputs["encoder_out"], dtype=np.float32)
    mask = np.asarray(inputs["src_mask"])
    hid = np.asarray(inputs["hiddens"], dtype=np.float32)
    cells = np.asarray(inputs["cells"], dtype=np.float32)
    W_ih = np.asarray(inputs["W_ih"], dtype=np.float32)
    W_hh = np.asarray(inputs["W_hh"], dtype=np.float32)
    b_ih = np.asarray(inputs["b_ih"], dtype=np.float32)
    b_hh = np.asarray(inputs["b_hh"], dtype=np.float32)
    W_in = np.asarray(inputs["W_in"], dtype=np.float32)
    b_in = np.asarray(inputs["b_in"], dtype=np.float32)
    W_out = np.asarray(inputs["W_out"], dtype=np.float32)
    b_out = np.asarray(inputs["b_out"], dtype=np.float32)

    def bf(x):
        return np.ascontiguousarray(x, dtype=NPBF)

    WIH = []
    WHH = []
    gscale = np.ones(G, np.float32)
    for nb in range(4):
        gscale[512 * nb + 384:512 * (nb + 1)] = 2.0   # tanh(g)=2*sig(2g)-1
    for l in range(L):
        wihT = W_ih[l].T[:, perm] * gscale
        biasrow = ((b_ih[l] + b_hh[l])[perm] * gscale)[None, :]
        WIH.append(bf(np.concatenate([wihT, biasrow], 0)))   # [513, 2048]
        WHH.append(bf(W_hh[l].T[:, perm] * gscale))          # [512, 2048]
    WINT = bf(W_in.T)                                        # [512, 1024]
    WOUTT = bf(np.concatenate([W_out.T, b_out[None, :]], 0))  # [1537, 512]

    # xpart0 injection selectors, one per u = t%8:
    # XPsb0 partition (4j+b)*8+u feeds gates row 32j+b
    einj8 = np.zeros((8, 128, 128), np.float32)
    for u in range(8):
        for j in range(4):
            for b in range(BS):
                einj8[u, (4 * j + b) * 8 + u, 32 * j + b] = 1.0
    einj8 = bf(einj8.transpose(1, 0, 2).reshape(128, 8 * 128))

    # block selector: e4blk[j, p] = 1 iff p // 32 == j
    e4 = np.zeros((4, 128), np.float32)
    for j in range(4):
        e4[j, 32 * j:32 * (j + 1)] = 1.0
    e4 = bf(e4)

    in_maps = []
    for core in range(NCORES):
        bsl = slice(core * BS, (core + 1) * BS)
        xe = embed[tokens[bsl]]                              # [BS, T, D]
        Xaug = np.concatenate(
            [xe.reshape(R, D), np.ones((R, 1), np.float32)], axis=1)
        # permute rows so phase-A store DMAs land partition-parallel:
        # new row b*64 + (t%8)*8 + t//8  <- (b, t)
        rperm = np.zeros(R, np.int64)
        for b in range(BS):
            for t in range(T):
                rperm[b * T + (t % 8) * 8 + t // 8] = b * T + t
        XT0 = bf(Xaug[rperm].T)                              # [513, 256]
        enc_c = np.ascontiguousarray(enc[bsl])               # [BS, 128, 1024]
        encT_c = np.swapaxes(enc_c, 1, 2)                    # [BS, 1024, 128]
        offs = np.einsum("bsd,d->bs", enc_c, b_in) + np.where(mask[bsl], -1e9, 0.0)
        offs_rep = np.ascontiguousarray(
            np.broadcast_to(offs[:, None, :], (BS, T, S)), dtype=np.float32)
        # initial c2T: c2t0[l, p, 32k+b] = hid[l, b, 128k+p]
        th = hid[:, bsl].reshape(L, BS, 4, 128).transpose(0, 3, 2, 1)  # [L,128,4,BS]
        c2t0 = np.zeros((L, 128, 4, 32), np.float32)
        c2t0[:, :, :, 0:BS] = th
        c2t0 = bf(c2t0.reshape(L, 128, 128))
        # cells in partition layout: cellsp[l, 32j+b, p] = cells[l, b, 128j+p]
        tc_ = cells[:, bsl].reshape(L, BS, 4, 128).transpose(0, 2, 1, 3)  # [L,4,BS,128]
        cellsp = np.zeros((L, 4, 32, 128), np.float32)
        cellsp[:, :, 0:BS, :] = tc_
        cellsp = bf(cellsp.reshape(L, 128, 128))
        in_maps.append({
            "xt0": XT0,
            "wih0": WIH[0], "whh0": WHH[0],
            "wih1": WIH[1], "whh1": WHH[1],
            "wint": WINT, "woutt": WOUTT,
            "enc": bf(enc_c), "enct": bf(encT_c), "offs": offs_rep,
            "c2t0": c2t0, "cellsp": cellsp,
            "ones1": np.ones((1, R), NPBF),
            "einj8": einj8, "e4blk": e4,
        })
    return in_maps


# ------------------------------------------------------------- device build

def build_program():
    nc = bacc.Bacc("TRN2", target_bir_lowering=False, debug=False)

    XT0 = nc.dram_tensor("xt0", [513, R], BF16, kind="ExternalInput")
    WIH0 = nc.dram_tensor("wih0", [513, G], BF16, kind="ExternalInput")
    WHH0 = nc.dram_tensor("whh0", [D, G], BF16, kind="ExternalInput")
    WIH1 = nc.dram_tensor("wih1", [513, G], BF16, kind="ExternalInput")
    WHH1 = nc.dram_tensor("whh1", [D, G], BF16, kind="ExternalInput")
    WINT = nc.dram_tensor("wint", [D, DS], BF16, kind="ExternalInput")
    WOUTT = nc.dram_tensor("woutt", [DS + D + 1, D], BF16, kind="ExternalInput")
    ENC = nc.dram_tensor("enc", [BS, S, DS], BF16, kind="ExternalInput")
    ENCT = nc.dram_tensor("enct", [BS, DS, S], BF16, kind="ExternalInput")
    OFFS = nc.dram_tensor("offs", [BS, T, S], F32, kind="ExternalInput")
    C2T0 = nc.dram_tensor("c2t0", [L, 128, 128], BF16, kind="ExternalInput")
    CELLSP = nc.dram_tensor("cellsp", [L, 128, 128], BF16, kind="ExternalInput")
    ONES1 = nc.dram_tensor("ones1", [1, R], BF16, kind="ExternalInput")
    EINJ = nc.dram_tensor("einj8", [128, 8 * 128], BF16, kind="ExternalInput")
    E4BLK = nc.dram_tensor("e4blk", [4, 128], BF16, kind="ExternalInput")
    OUT = nc.dram_tensor("out", [BS, T, D], F32, kind="ExternalOutput")

    with tile.TileContext(nc) as tc, ExitStack() as ctx:
        cpool = ctx.enter_context(tc.tile_pool(name="const", bufs=1))
        ident = cpool.tile([128, 128], F32)
        masks.make_identity(nc, ident[:])
        identb = cpool.tile([128, 128], BF16, name="identb")
        masks.make_identity(nc, identb[:])
        ones = cpool.tile([1, R], BF16, name="ones")
        nc.sync.dma_start(ones[:], ONES1.ap())
        einj8 = cpool.tile([128, 8 * 128], BF16, name="einj8")
        nc.sync.dma_start(einj8[:], EINJ.ap())
        e4blk = cpool.tile([4, 128], BF16, name="e4blk")
        nc.sync.dma_start(e4blk[:], E4BLK.ap())
        ones128 = cpool.tile([128, 128], BF16, name="ones128")
        nc.gpsimd.memset(ones128[:], 1.0)

        psp = ctx.enter_context(tc.tile_pool(name="ps", bufs=1, space="PSUM"))

        def gtile(idx, shape):
            return psp.tile(shape, F32, tag=f"g{idx}", name=f"g{idx}",
                            bufs=2 if idx < 2 else 1)

        # persistent SBUF xpart0:
        # XPsb0[(4*nb+b)*8 + t%8, (t//8)*512 + c] = xpart0[b,t,512nb+c]
        xpp = ctx.enter_context(tc.tile_pool(name="xps", bufs=1))
        XPsb0 = xpp.tile([128, (T // 8) * 512], BF16, name="xpsb0")

        # transposed h2 history per layer: hT[p, k*256 + b*64 + t]
        hT = [xpp.tile([128, 4 * R], BF16, name=f"hT{l}") for l in range(L)]

        # ---------------- Phase A inputs (packed, few DMAs) ----------------
        pa = ctx.enter_context(tc.tile_pool(name="pa", bufs=1))
        xtt = pa.tile([128, 4 * R], BF16, tag="xtt", name="xtt")
        nc.sync.dma_start(
            xtt[:].rearrange("p (k c) -> p k c", k=4),
            XT0.ap()[0:512].rearrange("(k p) c -> p k c", k=4))
        xt4 = pa.tile([1, R], BF16, tag="xt4", name="xt4")
        nc.sync.dma_start(xt4[:], XT0.ap()[512:513, :])
        wkt = pa.tile([128, 4 * G], BF16, tag="wkt", name="wkt")
        nc.sync.dma_start(
            wkt[:].rearrange("p (k c) -> p k c", k=4),
            WIH0.ap()[0:512].rearrange("(k p) c -> p k c", k=4))
        wk4 = pa.tile([1, G], BF16, tag="wk4", name="wk4")
        nc.sync.dma_start(wk4[:], WIH0.ap()[512:513, :])

        # PE warm-up: dummy matmuls on the identity while DMAs land
        wps = psp.tile([128, 128], F32, tag="g0", name="g0", bufs=2)
        for w in range(48):
            nc.tensor.matmul(wps[:], identb[:], identb[:],
                             start=True, stop=True, skip_group_check=True)

        # prefetch pool: recurrence weights + attention operands
        pf = ctx.enter_context(tc.tile_pool(name="pf", bufs=1))
        cTb = pf.tile([128, 2 * 128], BF16, tag="cTb", name="cTb")
        nc.sync.dma_start(
            cTb[:].rearrange("p (l c) -> p l c", l=2),
            CELLSP.ap().rearrange("l p c -> p l c"))
        c2T0b = pf.tile([128, 2 * 128], BF16, tag="c2T0b", name="c2T0b")
        nc.sync.dma_start(
            c2T0b[:].rearrange("p (l c) -> p l c", l=2),
            C2T0.ap().rearrange("l p c -> p l c"))
        whht = []
        for l, Wd in ((0, WHH0), (1, WHH1)):
            wt = pf.tile([128, 4 * G], BF16, tag=f"whh{l}", name=f"whh{l}")
            nc.sync.dma_start(
                wt[:].rearrange("p (k c) -> p k c", k=4),
                Wd.ap().rearrange("(k p) c -> p k c", k=4))
            whht.append(wt)
        wihC = pf.tile([128, 4 * G], BF16, tag="wihC", name="wihC")
        nc.sync.dma_start(
            wihC[:].rearrange("p (k c) -> p k c", k=4),
            WIH1.ap()[0:512].rearrange("(k p) c -> p k c", k=4))
        wihC4 = pf.tile([4, 512], BF16, tag="wihC4", name="wihC4")
        nc.sync.dma_start(wihC4[:], WIH1.ap()[512:513, :].rearrange(
            "a (j c) -> (a j) c", j=4))
        wint = pf.tile([128, 4 * DS], BF16, tag="wint", name="wint")
        nc.sync.dma_start(
            wint[:].rearrange("p (k c) -> p k c", k=4),
            WINT.ap().rearrange("(k p) c -> p k c", k=4))
        encb = pf.tile([S, 4 * DS], BF16, tag="encb", name="encb")
        nc.sync.dma_start(
            encb[:].rearrange("p (b c) -> p b c", b=BS),
            ENC.ap().rearrange("b s d -> s b d"))
        enctb = pf.tile([128, BS * 8 * S], BF16, tag="enctb", name="enctb")
        nc.sync.dma_start(
            enctb[:].rearrange("p (b k s) -> p b k s", b=BS, k=8),
            ENCT.ap().rearrange("b (k p) s -> p b k s", k=8))
        offsb = pf.tile([T, BS * S], F32, tag="offsb", name="offsb")
        nc.sync.dma_start(
            offsb[:].rearrange("p (b s) -> p b s", b=BS),
            OFFS.ap().rearrange("b t s -> t b s"))
        woutt = pf.tile([128, 12 * D], BF16, tag="woutt", name="woutt")
        nc.sync.dma_start(
            woutt[:].rearrange("p (k c) -> p k c", k=12),
            WOUTT.ap()[0:1536].rearrange("(k p) c -> p k c", k=12))
        woutb = pf.tile([1, D], BF16, tag="woutb", name="woutb")
        nc.sync.dma_start(woutb[:], WOUTT.ap()[1536:1537, :])

        # ---------------- Phase A: xpart0 ----------------
        for mc in range(2):
            for nb in range(4):
                ps = gtile(nb % 2, [128, 512])
                for k in range(4):
                    nc.tensor.matmul(
                        ps[:],
                        xtt[:, 256 * k + 128 * mc:256 * k + 128 * (mc + 1)],
                        wkt[:, 2048 * k + 512 * nb:2048 * k + 512 * (nb + 1)],
                        start=(k == 0), stop=False)
                nc.tensor.matmul(
                    ps[:], xt4[:, 128 * mc:128 * (mc + 1)],
                    wk4[:, 512 * nb:512 * (nb + 1)],
                    start=False, stop=True)
                sb = pa.tile([128, 512], BF16, tag=f"stg{nb}", name=f"stg{nb}")
                nc.scalar.copy(sb[:], ps[:])
                p0 = (4 * nb + 2 * mc) * 8
                dst = XPsb0[p0:p0 + 16, :].rearrange(
                    "p (q c) -> p q c", c=512)
                nc.sync.dma_start(dst, sb[:])

        # ---------------- Interleaved recurrence passes ----------------
        rp = ctx.enter_context(tc.tile_pool(name="rp", bufs=2))

        def linit(l):
            return {"l": l, "cT": cTb[:, 128 * l:128 * (l + 1)],
                    "c2T": c2T0b[:, 128 * l:128 * (l + 1)], "whh": whht[l],
                    "sall_prev": None, "c2h_prev": None, "tprev": -1}

        def lstep_mm(st, t):
            """Inject/xpart + W_hh rounds for step t (PE bulk)."""
            l = st["l"]
            gates = gtile(l, [128, 512])
            if l == 0:
                rhs = XPsb0[:, 512 * (t // 8):512 * (t // 8 + 1)]
                nc.tensor.matmul(
                    gates[:], einj8[:, 128 * (t % 8):128 * (t % 8 + 1)], rhs,
                    start=True, stop=False, skip_group_check=True)
            else:
                # xpart1(t) accumulated in place: bias row first (writes all
                # 128 partitions), then W_ih1 rounds from hT0 columns of t
                nc.tensor.matmul(gates[:], e4blk[:], wihC4[:],
                                 start=True, stop=False, skip_group_check=True)
                for k in range(4):
                    lhsT = hT[0][:].rearrange(
                        "p (k b t) -> p k b t", k=4, b=BS)[:, k, :, t]
                    for j in range(4):
                        nc.tensor.matmul(
                            gates[32 * j:32 * j + BS, :],
                            lhsT,
                            wihC[:, 2048 * k + 512 * j:2048 * k + 512 * (j + 1)],
                            start=False, stop=False,
                            tile_position=(0, 32 * j), skip_group_check=True)
            for k in range(4):
                lhsT = st["c2T"][:, 32 * k:32 * k + BS]
                for j in range(4):
                    nc.tensor.matmul(
                        gates[32 * j:32 * j + BS, :],
                        lhsT,
                        st["whh"][:, 2048 * k + 512 * j:2048 * k + 512 * (j + 1)],
                        start=False, stop=(k == 3),
                        tile_position=(0, 32 * j), skip_group_check=True)
            st["gates"] = gates

        def lstep_tail(st, t):
            """sigma + c2 chain + transpose + cast for step t."""
            l = st["l"]
            gates = st["gates"]
            sall = rp.tile([128, 512], F32, tag=f"sa{l}", name=f"sa{l}")
            nc.scalar.activation(sall[:], gates[:], AF.Sigmoid)
            m1 = rp.tile([128, 128], BF16, tag=f"m1{l}", name=f"m1{l}")
            nc.vector.tensor_mul(m1[:], sall[:, 128:256], st["cT"])
            tgv = rp.tile([128, 128], BF16, tag=f"tg{l}", name=f"tg{l}")
            nc.vector.scalar_tensor_tensor(
                tgv[:], sall[:, 384:512], 2.0, ones128[:],
                mybir.AluOpType.mult, mybir.AluOpType.subtract)
            m2 = rp.tile([128, 128], BF16, tag=f"m2{l}", name=f"m2{l}")
            nc.vector.tensor_mul(m2[:], sall[:, 0:128], tgv[:])
            c2h = rp.tile([128, 128], BF16, tag=f"c2h{l}", name=f"c2h{l}")
            nc.vector.tensor_add(c2h[:], m1[:], m2[:])
            tp = psp.tile([128, 128], BF16, tag=f"tp{l}", name=f"tp{l}",
                          bufs=2)
            nc.tensor.transpose(tp[:], c2h[:], identb[:])
            c2T_new = rp.tile([128, 128], BF16, tag=f"c2T{l}", name=f"c2T{l}")
            nc.vector.tensor_copy(c2T_new[:], tp[:])
            st["sall"] = sall
            st["c2h"] = c2h
            st["c2T"] = c2T_new

        def lstep_branch(st):
            """Delayed h2 branch for the PREVIOUS step (never blocks chains)."""
            l = st["l"]
            if st["sall_prev"] is not None:
                tc2 = rp.tile([128, 128], BF16, tag=f"tc2{l}", name=f"tc2{l}")
                nc.scalar.activation(tc2[:], st["c2h_prev"][:], AF.Tanh)
                h2 = rp.tile([128, 128], BF16, tag=f"h2{l}", name=f"h2{l}")
                nc.gpsimd.tensor_mul(h2[:], st["sall_prev"][:, 256:384], tc2[:])
                tp2 = psp.tile([128, 128], BF16, tag=f"tp{l}", name=f"tp{l}",
                               bufs=2)
                nc.tensor.transpose(tp2[:], h2[:], identb[:])
                src_ = tp2[:].rearrange("p (k r) -> p k r", k=4)[:, :, 0:BS]
                dst = hT[l][:].rearrange(
                    "p (k b t) -> p k b t", k=4, b=BS)[:, :, :, st["tprev"]]
                nc.vector.tensor_copy(dst, src_)
            st["sall_prev"] = st["sall"]
            st["c2h_prev"] = st["c2h"]
            st["tprev"] = st["tprev"] + 1

        st0 = linit(0)
        st1 = linit(1)
        for ss in range(T + LAG + 1):
            if ss < T:
                lstep_mm(st0, ss)
            if LAG <= ss < T + LAG:
                lstep_mm(st1, ss - LAG)
            if ss < T:
                lstep_tail(st0, ss)
            if LAG <= ss < T + LAG:
                lstep_tail(st1, ss - LAG)
            if ss <= T:
                lstep_branch(st0)
            if LAG <= ss <= T + LAG:
                lstep_branch(st1)

        # ---------------- Phase E: attention + out proj ----------------
        wkt2 = pa.tile([128, 4 * G], BF16, tag="wkt", name="wkt2")
        with tc.tile_pool(name="pe", bufs=1) as pe:
            sT = [hT[1][:, 256 * k:256 * (k + 1)] for k in range(4)]

            xqT = []
            for m in range(8):
                ps = gtile(m % 2, [128, R])
                for k in range(4):
                    nc.tensor.matmul(
                        ps[:], wint[:, 1024 * k + 128 * m:1024 * k + 128 * (m + 1)],
                        sT[k], start=(k == 0), stop=(k == 3))
                xq = wkt2[:, 256 * m:256 * (m + 1)]
                if m % 2 == 0:
                    nc.scalar.copy(xq, ps[:])
                else:
                    nc.vector.tensor_copy(xq, ps[:])
                xqT.append(xq)

            ctxT = [wkt2[:, 2048 + 256 * m:2048 + 256 * (m + 1)]
                    for m in range(8)]
            for b in range(BS):
                bsl = slice(T * b, T * (b + 1))
                eps = psp.tile([T, S], F32, tag=f"g{b % 2}",
                               name=f"g{b % 2}", bufs=2)
                for k in range(8):
                    nc.tensor.matmul(
                        eps[:], xqT[k][:, bsl],
                        enctb[:, 1024 * b + 128 * k:1024 * b + 128 * (k + 1)],
                        start=(k == 0), stop=(k == 7))
                esb = pe.tile([T, S], F32, tag=f"esb{b % 2}",
                              name=f"esb{b % 2}")
                nc.vector.tensor_add(esb[:], eps[:],
                                     offsb[:, 128 * b:128 * (b + 1)])
                negmax = pe.tile([T, 1], F32, tag=f"negmax{b % 2}",
                                 name=f"negmax{b % 2}")
                nc.vector.reduce_max(
                    negmax[:], esb[:], axis=mybir.AxisListType.X, negate=True)
                expE = pe.tile([T, S], F32, tag=f"expE{b % 2}",
                               name=f"expE{b % 2}")
                den = pe.tile([T, 1], F32, tag=f"den{b % 2}",
                              name=f"den{b % 2}")
                nc.scalar.activation(
                    expE[:], esb[:], AF.Exp, bias=negmax[:], accum_out=den[:])
                rden = pe.tile([T, 1], F32, tag=f"rden{b % 2}",
                               name=f"rden{b % 2}")
                nc.vector.reciprocal(rden[:], den[:])
                attn = pe.tile([T, S], F32, tag=f"attn{b % 2}",
                               name=f"attn{b % 2}")
                nc.vector.tensor_scalar_mul(attn[:], expE[:], rden[:])
                tpa = psp.tile([S, T], F32, tag=f"tp{b % 2}",
                               name=f"tp{b % 2}", bufs=2)
                nc.tensor.transpose(tpa[:], attn[:], ident[0:T, 0:T])
                atsb = pe.tile([S, T], BF16, tag=f"atsb{b % 2}",
                               name=f"atsb{b % 2}")
                nc.vector.tensor_copy(atsb[:], tpa[:])
                for m in range(8):
                    psc = gtile(m % 2, [128, T])
                    nc.tensor.matmul(
                        psc[:], encb[:, 1024 * b + 128 * m:1024 * b + 128 * (m + 1)],
                        atsb[:], start=True, stop=True)
                    if m % 2 == 0:
                        nc.scalar.copy(ctxT[m][:, bsl], psc[:])
                    else:
                        nc.vector.tensor_copy(ctxT[m][:, bsl], psc[:])

            outflat = OUT.ap().rearrange("b t d -> (b t) d")
            lhs_all = ctxT + sT + [ones]
            wt_all = [woutt[:, 512 * k:512 * (k + 1)] for k in range(12)] \
                + [woutb[:]]
            for mc in range(2):
                msl = slice(128 * mc, 128 * (mc + 1))
                ps = gtile(mc, [128, D])
                for k in range(13):
                    nc.tensor.matmul(
                        ps[:], lhs_all[k][:, msl], wt_all[k],
                        start=(k == 0), stop=(k == 12))
                osb = pa.tile([128, D], F32, tag=f"stg{mc}", name=f"osb{mc}")
                nc.scalar.activation(osb[:], ps[:], AF.Tanh)
                nc.sync.dma_start(outflat[msl, :], osb[:])

    nc.compile()
    return nc


def assemble(results):
    full = np.concatenate([r["out"] for r in results], axis=0)  # [B, T, D]
    outs = full.transpose(1, 0, 2)                              # [T, B, D]
    return np.ascontiguousarray(outs.reshape(-1, D).reshape(-1, T, D))


_nc_cache = None


def kernel(**inputs):
    global _nc_cache
    in_maps = host_prep(inputs)
    if _nc_cache is None:
        _nc_cache = build_program()
    res = run_bass_kernel_spmd(_nc_cache, in_maps, list(range(NCORES)))
    return assemble(res.results)


# revision 21
# speedup vs baseline: 1.9689x; 1.0211x over previous
"""Trainium2 Bass kernel for nn_Decoder_46042049413334.

Buggy 2-layer LSTM decoder with attention (B=32, T=64, S=128, D=512).

Structure (per core, batch sharded 8 ways, BS=4 examples/core):
  Phase A: xpart0 = [emb(tokens), 1] @ [W_ih0.T; b0]  -> XPsb0 (SBUF)
  Interleaved pass: layer-0 step t and layer-1 step t-2 run together;
    layer-1's xpart is accumulated per step directly into its gates
    PSUM from the transposed h2 history (hT0), so the PE stays busy
    enough to hold the HAM clock gate open (K=8/8).
  Phase E: attention + out-projection from hT1

Recurrence layout: gates PSUM [128, 512] where partition 32*j+b holds
(example b, d-block j) and the 512 free cols are {i,f,o,2g}x128 for
that d-block (g columns pre-scaled by 2 so tanh(g) = 2*sigmoid(2g)-1
comes out of a single full-width sigmoid). The four d-blocks' weight
streams run CONCURRENTLY in the PE array via tile_position=(0, 32*j)
column tiling. Elementwise runs once over all 128 partitions; c2 and
h2 are re-transposed per step ([128,128] PE transpose). Off-chain work
(tanh(c2), h2, its transpose/gather) is emitted one step late so no
engine FIFO ever blocks the recurrence chains.

Row ordering is b-major everywhere: row r = b_local*T + t.
"""
import numpy as np
import ml_dtypes
from contextlib import ExitStack

import concourse.bass as bass
import concourse.bacc as bacc
import concourse.tile as tile
from concourse import mybir, masks
from concourse.bass_utils import run_bass_kernel_spmd

F32 = mybir.dt.float32
BF16 = mybir.dt.bfloat16
AF = mybir.ActivationFunctionType
NPBF = ml_dtypes.bfloat16

B, T, S, D, L, V = 32, 64, 128, 512, 2, 32000
G = 4 * D        # 2048
DS = 2 * D       # 1024
NCORES = 8
BS = B // NCORES  # 4
R = BS * T        # 256 rows per core
LAG = 2          # layer-1 recurrence lag behind layer 0


# ---------------------------------------------------------------- host side

def _gate_perm():
    perm = np.zeros(G, dtype=np.int64)
    base = {0: 0, 1: 512, 2: 1536, 3: 1024}  # i, f, o, g
    for j in range(G):
        nb, pos = divmod(j, 512)
        sub, dd = divmod(pos, 128)
        perm[j] = base[sub] + nb * 128 + dd
    return perm


def host_prep(inputs):
    """Build the 8 per-core input maps (layout/gather work only)."""
    perm = _gate_perm()
    tokens = np.asarray(inputs["prev_tgt_tokens"])
    embed = np.asarray(inputs["embed"], dtype=np.float32)
    enc = np.asarray(inputs["encoder_out"], dtype=np.float32)
    mask = np.asarray(inputs["src_mask"])
    hid = np.asarray(inputs["hiddens"], dtype=np.float32)
    cells = np.asarray(inputs["cells"], dtype=np.float32)
    W_ih = np.asarray(inputs["W_ih"], dtype=np.float32)
    W_hh = np.asarray(inputs["W_hh"], dtype=np.float32)
    b_ih = np.asarray(inputs["b_ih"], dtype=np.float32)
    b_hh = np.asarray(inputs["b_hh"], dtype=np.float32)
    W_in = np.asarray(inputs["W_in"], dtype=np.float32)
    b_in = np.asarray(inputs["b_in"], dtype=np.float32)
    W_out = np.asarray(inputs["W_out"], dtype=np.float32)
    b_out = np.asarray(inputs["b_out"], dtype=np.float32)

    def bf(x):
        return np.ascontiguousarray(x, dtype=NPBF)

    WIH = []
    WHH = []
    gscale = np.ones(G, np.float32)
    for nb in range(4):
        gscale[512 * nb + 384:512 * (nb + 1)] = 2.0   # tanh(g)=2*sig(2g)-1
    for l in range(L):
        wihT = W_ih[l].T[:, perm] * gscale
        biasrow = ((b_ih[l] + b_hh[l])[perm] * gscale)[None, :]
        WIH.append(bf(np.concatenate([wihT, biasrow], 0)))   # [513, 2048]
        WHH.append(bf(W_hh[l].T[:, perm] * gscale))          # [512, 2048]
    WINT = bf(W_in.T)                                        # [512, 1024]
    WOUTT = bf(np.concatenate([W_out.T, b_out[None, :]], 0))  # [1537, 512]

    # xpart0 injection selectors, one per u = t%8:
    # XPsb0 partition (4j+b)*8+u feeds gates row 32j+b
    einj8 = np.zeros((8, 128, 128), np.float32)
    for u in range(8):
        for j in range(4):
            for b in range(BS):
                einj8[u, (4 * j + b) * 8 + u, 32 * j + b] = 1.0
    einj8 = bf(einj8.transpose(1, 0, 2).reshape(128, 8 * 128))

    # block selector: e4blk[j, p] = 1 iff p // 32 == j
    e4 = np.zeros((4, 128), np.float32)
    for j in range(4):
        e4[j, 32 * j:32 * (j + 1)] = 1.0
    e4 = bf(e4)

    in_maps = []
    for core in range(NCORES):
        bsl = slice(core * BS, (core + 1) * BS)
        xe = embed[tokens[bsl]]                              # [BS, T, D]
        Xaug = np.concatenate(
            [xe.reshape(R, D), np.ones((R, 1), np.float32)], axis=1)
        # permute rows so phase-A store DMAs land partition-parallel:
        # new row b*64 + (t%8)*8 + t//8  <- (b, t)
        rperm = np.zeros(R, np.int64)
        for b in range(BS):
            for t in range(T):
                rperm[b * T + (t % 8) * 8 + t // 8] = b * T + t
        XT0 = bf(Xaug[rperm].T)                              # [513, 256]
        enc_c = np.ascontiguousarray(enc[bsl])               # [BS, 128, 1024]
        encT_c = np.swapaxes(enc_c, 1, 2)                    # [BS, 1024, 128]
        offs = np.einsum("bsd,d->bs", enc_c, b_in) + np.where(mask[bsl], -1e9, 0.0)
        offs_rep = np.ascontiguousarray(
            np.broadcast_to(offs[:, None, :], (BS, T, S)), dtype=np.float32)
        # initial c2T: c2t0[l, p, 32k+b] = hid[l, b, 128k+p]
        th = hid[:, bsl].reshape(L, BS, 4, 128).transpose(0, 3, 2, 1)  # [L,128,4,BS]
        c2t0 = np.zeros((L, 128, 4, 32), np.float32)
        c2t0[:, :, :, 0:BS] = th
        c2t0 = bf(c2t0.reshape(L, 128, 128))
        # cells in partition layout: cellsp[l, 32j+b, p] = cells[l, b, 128j+p]
        tc_ = cells[:, bsl].reshape(L, BS, 4, 128).transpose(0, 2, 1, 3)  # [L,4,BS,128]
        cellsp = np.zeros((L, 4, 32, 128), np.float32)
        cellsp[:, :, 0:BS, :] = tc_
        cellsp = bf(cellsp.reshape(L, 128, 128))
        in_maps.append({
            "xt0": XT0,
            "wih0": WIH[0], "whh0": WHH[0],
            "wih1": WIH[1], "whh1": WHH[1],
            "wint": WINT, "woutt": WOUTT,
            "enc": bf(enc_c), "enct": bf(encT_c), "offs": offs_rep,
            "c2t0": c2t0, "cellsp": cellsp,
            "ones1": np.ones((1, R), NPBF),
            "einj8": einj8, "e4blk": e4,
        })
    return in_maps


# ------------------------------------------------------------- device build

def build_program():
    nc = bacc.Bacc("TRN2", target_bir_lowering=False, debug=False)

    XT0 = nc.dram_tensor("xt0", [513, R], BF16, kind="ExternalInput")
    WIH0 = nc.dram_tensor("wih0", [513, G], BF16, kind="ExternalInput")
    WHH0 = nc.dram_tensor("whh0", [D, G], BF16, kind="ExternalInput")
    WIH1 = nc.dram_tensor("wih1", [513, G], BF16, kind="ExternalInput")
    WHH1 = nc.dram_tensor("whh1", [D, G], BF16, kind="ExternalInput")
    WINT = nc.dram_tensor("wint", [D, DS], BF16, kind="ExternalInput")
    WOUTT = nc.dram_tensor("woutt", [DS + D + 1, D], BF16, kind="ExternalInput")
    ENC = nc.dram_tensor("enc", [BS, S, DS], BF16, kind="ExternalInput")
    ENCT = nc.dram_tensor("enct", [BS, DS, S], BF16, kind="ExternalInput")
    OFFS = nc.dram_tensor("offs", [BS, T, S], F32, kind="ExternalInput")
    C2T0 = nc.dram_tensor("c2t0", [L, 128, 128], BF16, kind="ExternalInput")
    CELLSP = nc.dram_tensor("cellsp", [L, 128, 128], BF16, kind="ExternalInput")
    ONES1 = nc.dram_tensor("ones1", [1, R], BF16, kind="ExternalInput")
    EINJ = nc.dram_tensor("einj8", [128, 8 * 128], BF16, kind="ExternalInput")
    E4BLK = nc.dram_tensor("e4blk", [4, 128], BF16, kind="ExternalInput")
    OUT = nc.dram_tensor("out", [BS, T, D], F32, kind="ExternalOutput")

    with tile.TileContext(nc) as tc, ExitStack() as ctx:
        cpool = ctx.enter_context(tc.tile_pool(name="const", bufs=1))
        ident = cpool.tile([128, 128], F32)
        masks.make_identity(nc, ident[:])
        identb = cpool.tile([128, 128], BF16, name="identb")
        masks.make_identity(nc, identb[:])
        ones = cpool.tile([1, R], BF16, name="ones")
        nc.sync.dma_start(ones[:], ONES1.ap())
        einj8 = cpool.tile([128, 8 * 128], BF16, name="einj8")
        nc.sync.dma_start(einj8[:], EINJ.ap())
        e4blk = cpool.tile([4, 128], BF16, name="e4blk")
        nc.sync.dma_start(e4blk[:], E4BLK.ap())
        ones128 = cpool.tile([128, 128], BF16, name="ones128")
        nc.gpsimd.memset(ones128[:], 1.0)

        psp = ctx.enter_context(tc.tile_pool(name="ps", bufs=1, space="PSUM"))

        def gtile(idx, shape):
            return psp.tile(shape, F32, tag=f"g{idx}", name=f"g{idx}",
                            bufs=2 if idx < 2 else 1)

        # persistent SBUF xpart0:
        # XPsb0[(4*nb+b)*8 + t%8, (t//8)*512 + c] = xpart0[b,t,512nb+c]
        xpp = ctx.enter_context(tc.tile_pool(name="xps", bufs=1))
        XPsb0 = xpp.tile([128, (T // 8) * 512], BF16, name="xpsb0")

        # transposed h2 history per layer: hT[p, k*256 + b*64 + t]
        hT = [xpp.tile([128, 4 * R], BF16, name=f"hT{l}") for l in range(L)]

        # ---------------- Phase A inputs (packed, few DMAs) ----------------
        pa = ctx.enter_context(tc.tile_pool(name="pa", bufs=1))
        xtt = pa.tile([128, 4 * R], BF16, tag="xtt", name="xtt")
        nc.sync.dma_start(
            xtt[:].rearrange("p (k c) -> p k c", k=4),
            XT0.ap()[0:512].rearrange("(k p) c -> p k c", k=4))
        xt4 = pa.tile([1, R], BF16, tag="xt4", name="xt4")
        nc.sync.dma_start(xt4[:], XT0.ap()[512:513, :])
        wkt = pa.tile([128, 4 * G], BF16, tag="wkt", name="wkt")
        nc.sync.dma_start(
            wkt[:].rearrange("p (k c) -> p k c", k=4),
            WIH0.ap()[0:512].rearrange("(k p) c -> p k c", k=4))
        wk4 = pa.tile([1, G], BF16, tag="wk4", name="wk4")
        nc.sync.dma_start(wk4[:], WIH0.ap()[512:513, :])

        # PE warm-up: dummy matmuls on the identity while DMAs land
        wps = psp.tile([128, 128], F32, tag="g0", name="g0", bufs=2)
        for w in range(48):
            nc.tensor.matmul(wps[:], identb[:], identb[:],
                             start=True, stop=True, skip_group_check=True)

        # prefetch pool: recurrence weights + attention operands
        pf = ctx.enter_context(tc.tile_pool(name="pf", bufs=1))
        cTb = pf.tile([128, 2 * 128], BF16, tag="cTb", name="cTb")
        nc.sync.dma_start(
            cTb[:].rearrange("p (l c) -> p l c", l=2),
            CELLSP.ap().rearrange("l p c -> p l c"))
        c2T0b = pf.tile([128, 2 * 128], BF16, tag="c2T0b", name="c2T0b")
        nc.sync.dma_start(
            c2T0b[:].rearrange("p (l c) -> p l c", l=2),
            C2T0.ap().rearrange("l p c -> p l c"))
        whht = []
        for l, Wd in ((0, WHH0), (1, WHH1)):
            wt = pf.tile([128, 4 * G], BF16, tag=f"whh{l}", name=f"whh{l}")
            nc.sync.dma_start(
                wt[:].rearrange("p (k c) -> p k c", k=4),
                Wd.ap().rearrange("(k p) c -> p k c", k=4))
            whht.append(wt)
        wihC = pf.tile([128, 4 * G], BF16, tag="wihC", name="wihC")
        nc.sync.dma_start(
            wihC[:].rearrange("p (k c) -> p k c", k=4),
            WIH1.ap()[0:512].rearrange("(k p) c -> p k c", k=4))
        wihC4 = pf.tile([4, 512], BF16, tag="wihC4", name="wihC4")
        nc.sync.dma_start(wihC4[:], WIH1.ap()[512:513, :].rearrange(
            "a (j c) -> (a j) c", j=4))
        # ---------------- Phase A: xpart0 ----------------
        for mc in range(2):
            for nb in range(4):
                ps = gtile(nb % 2, [128, 512])
                for k in range(4):
                    nc.tensor.matmul(
                        ps[:],
                        xtt[:, 256 * k + 128 * mc:256 * k + 128 * (mc + 1)],
                        wkt[:, 2048 * k + 512 * nb:2048 * k + 512 * (nb + 1)],
                        start=(k == 0), stop=False)
                nc.tensor.matmul(
                    ps[:], xt4[:, 128 * mc:128 * (mc + 1)],
                    wk4[:, 512 * nb:512 * (nb + 1)],
                    start=False, stop=True)
                sb = pa.tile([128, 512], BF16, tag=f"stg{nb}", name=f"stg{nb}")
                nc.scalar.copy(sb[:], ps[:])
                p0 = (4 * nb + 2 * mc) * 8
                dst = XPsb0[p0:p0 + 16, :].rearrange(
                    "p (q c) -> p q c", c=512)
                nc.sync.dma_start(dst, sb[:])

        # ---------------- Interleaved recurrence passes ----------------
        rp = ctx.enter_context(tc.tile_pool(name="rp", bufs=2))

        def linit(l):
            return {"l": l, "cT": cTb[:, 128 * l:128 * (l + 1)],
                    "c2T": c2T0b[:, 128 * l:128 * (l + 1)], "whh": whht[l],
                    "sall_prev": None, "c2h_prev": None, "tprev": -1}

        def lstep_mm(st, t):
            """Inject/xpart + W_hh rounds for step t (PE bulk)."""
            l = st["l"]
            gates = gtile(l, [128, 512])
            if l == 0:
                rhs = XPsb0[:, 512 * (t // 8):512 * (t // 8 + 1)]
                nc.tensor.matmul(
                    gates[:], einj8[:, 128 * (t % 8):128 * (t % 8 + 1)], rhs,
                    start=True, stop=False, skip_group_check=True)
            else:
                # xpart1(t) accumulated in place: bias row first (writes all
                # 128 partitions), then W_ih1 rounds from hT0 columns of t
                nc.tensor.matmul(gates[:], e4blk[:], wihC4[:],
                                 start=True, stop=False, skip_group_check=True)
                for k in range(4):
                    lhsT = hT[0][:].rearrange(
                        "p (k b t) -> p k b t", k=4, b=BS)[:, k, :, t]
                    for j in range(4):
                        nc.tensor.matmul(
                            gates[32 * j:32 * j + BS, :],
                            lhsT,
                            wihC[:, 2048 * k + 512 * j:2048 * k + 512 * (j + 1)],
                            start=False, stop=False,
                            tile_position=(0, 32 * j), skip_group_check=True)
            for k in range(4):
                lhsT = st["c2T"][:, 32 * k:32 * k + BS]
                for j in range(4):
                    nc.tensor.matmul(
                        gates[32 * j:32 * j + BS, :],
                        lhsT,
                        st["whh"][:, 2048 * k + 512 * j:2048 * k + 512 * (j + 1)],
                        start=False, stop=(k == 3),
                        tile_position=(0, 32 * j), skip_group_check=True)
            st["gates"] = gates

        def lstep_tail(st, t):
            """sigma + c2 chain + transpose + cast for step t."""
            l = st["l"]
            gates = st["gates"]
            sall = rp.tile([128, 512], F32, tag=f"sa{l}", name=f"sa{l}",
                           bufs=3)
            nc.scalar.activation(sall[:], gates[:], AF.Sigmoid)
            m1 = rp.tile([128, 128], BF16, tag=f"m1{l}", name=f"m1{l}")
            nc.gpsimd.tensor_mul(m1[:], sall[:, 128:256], st["cT"])
            tgv = rp.tile([128, 128], BF16, tag=f"tg{l}", name=f"tg{l}")
            nc.vector.scalar_tensor_tensor(
                tgv[:], sall[:, 384:512], 2.0, ones128[:],
                mybir.AluOpType.mult, mybir.AluOpType.subtract)
            m2 = rp.tile([128, 128], BF16, tag=f"m2{l}", name=f"m2{l}")
            nc.vector.tensor_mul(m2[:], sall[:, 0:128], tgv[:])
            c2h = rp.tile([128, 128], BF16, tag=f"c2h{l}", name=f"c2h{l}",
                          bufs=3)
            nc.vector.tensor_add(c2h[:], m1[:], m2[:])
            tp = psp.tile([128, 128], BF16, tag=f"tp{l}", name=f"tp{l}",
                          bufs=2)
            nc.tensor.transpose(tp[:], c2h[:], identb[:])
            c2T_new = rp.tile([128, 128], BF16, tag=f"c2T{l}", name=f"c2T{l}")
            nc.vector.tensor_copy(c2T_new[:], tp[:])
            st["sall"] = sall
            st["c2h"] = c2h
            st["c2T"] = c2T_new

        def lstep_branch(st):
            """Delayed h2 branch for the PREVIOUS step (never blocks chains)."""
            l = st["l"]
            if st["sall_prev"] is not None:
                tc2 = rp.tile([128, 128], BF16, tag=f"tc2{l}", name=f"tc2{l}")
                nc.scalar.activation(tc2[:], st["c2h_prev"][:], AF.Tanh)
                h2 = rp.tile([128, 128], BF16, tag=f"h2{l}", name=f"h2{l}")
                nc.gpsimd.tensor_mul(h2[:], st["sall_prev"][:, 256:384], tc2[:])
                tp2 = psp.tile([128, 128], BF16, tag=f"tp{l}", name=f"tp{l}",
                               bufs=2)
                nc.tensor.transpose(tp2[:], h2[:], identb[:])
                src_ = tp2[:].rearrange("p (k r) -> p k r", k=4)[:, :, 0:BS]
                dst = hT[l][:].rearrange(
                    "p (k b t) -> p k b t", k=4, b=BS)[:, :, :, st["tprev"]]
                nc.vector.tensor_copy(dst, src_)
            st["sall_prev"] = st["sall"]
            st["c2h_prev"] = st["c2h"]
            st["tprev"] = st["tprev"] + 1

        # phase-E operands: transferred during the recurrence pass
        # phase-E operands: transferred during the recurrence pass
        wint = pf.tile([128, 4 * DS], BF16, tag="wint", name="wint")
        nc.sync.dma_start(
            wint[:].rearrange("p (k c) -> p k c", k=4),
            WINT.ap().rearrange("(k p) c -> p k c", k=4))
        encb = pf.tile([S, 4 * DS], BF16, tag="encb", name="encb")
        nc.sync.dma_start(
            encb[:].rearrange("p (b c) -> p b c", b=BS),
            ENC.ap().rearrange("b s d -> s b d"))
        enctb = pf.tile([128, BS * 8 * S], BF16, tag="enctb", name="enctb")
        nc.sync.dma_start(
            enctb[:].rearrange("p (b k s) -> p b k s", b=BS, k=8),
            ENCT.ap().rearrange("b (k p) s -> p b k s", k=8))
        offsb = pf.tile([T, BS * S], F32, tag="offsb", name="offsb")
        nc.sync.dma_start(
            offsb[:].rearrange("p (b s) -> p b s", b=BS),
            OFFS.ap().rearrange("b t s -> t b s"))
        woutt = pf.tile([128, 12 * D], BF16, tag="woutt", name="woutt")
        nc.sync.dma_start(
            woutt[:].rearrange("p (k c) -> p k c", k=12),
            WOUTT.ap()[0:1536].rearrange("(k p) c -> p k c", k=12))
        woutb = pf.tile([1, D], BF16, tag="woutb", name="woutb")
        nc.sync.dma_start(woutb[:], WOUTT.ap()[1536:1537, :])

        st0 = linit(0)
        st1 = linit(1)
        for ss in range(T + LAG + 1):
            if ss < T:
                lstep_mm(st0, ss)
            if LAG <= ss < T + LAG:
                lstep_mm(st1, ss - LAG)
            if ss < T:
                lstep_tail(st0, ss)
            if LAG <= ss < T + LAG:
                lstep_tail(st1, ss - LAG)
            if ss <= T:
                lstep_branch(st0)
            if LAG <= ss <= T + LAG:
                lstep_branch(st1)

        # ---------------- Phase E: attention + out proj ----------------
        wkt2 = pa.tile([128, 4 * G], BF16, tag="wkt", name="wkt2")
        with tc.tile_pool(name="pe", bufs=1) as pe:
            sT = [hT[1][:, 256 * k:256 * (k + 1)] for k in range(4)]

            xqT = []
            for m in range(8):
                ps = gtile(m % 2, [128, R])
                for k in range(4):
                    nc.tensor.matmul(
                        ps[:], wint[:, 1024 * k + 128 * m:1024 * k + 128 * (m + 1)],
                        sT[k], start=(k == 0), stop=(k == 3))
                xq = wkt2[:, 256 * m:256 * (m + 1)]
                if m % 2 == 0:
                    nc.scalar.copy(xq, ps[:])
                else:
                    nc.vector.tensor_copy(xq, ps[:])
                xqT.append(xq)

            ctxT = [wkt2[:, 2048 + 256 * m:2048 + 256 * (m + 1)]
                    for m in range(8)]
            for b in range(BS):
                bsl = slice(T * b, T * (b + 1))
                eps = psp.tile([T, S], F32, tag=f"g{b % 2}",
                               name=f"g{b % 2}", bufs=2)
                for k in range(8):
                    nc.tensor.matmul(
                        eps[:], xqT[k][:, bsl],
                        enctb[:, 1024 * b + 128 * k:1024 * b + 128 * (k + 1)],
                        start=(k == 0), stop=(k == 7))
                esb = pe.tile([T, S], F32, tag=f"esb{b % 2}",
                              name=f"esb{b % 2}")
                nc.vector.tensor_add(esb[:], eps[:],
                                     offsb[:, 128 * b:128 * (b + 1)])
                negmax = pe.tile([T, 1], F32, tag=f"negmax{b % 2}",
                                 name=f"negmax{b % 2}")
                nc.vector.reduce_max(
                    negmax[:], esb[:], axis=mybir.AxisListType.X, negate=True)
                expE = pe.tile([T, S], F32, tag=f"expE{b % 2}",
                               name=f"expE{b % 2}")
                den = pe.tile([T, 1], F32, tag=f"den{b % 2}",
                              name=f"den{b % 2}")
                nc.scalar.activation(
                    expE[:], esb[:], AF.Exp, bias=negmax[:], accum_out=den[:])
                rden = pe.tile([T, 1], F32, tag=f"rden{b % 2}",
                               name=f"rden{b % 2}")
                nc.vector.reciprocal(rden[:], den[:])
                attn = pe.tile([T, S], F32, tag=f"attn{b % 2}",
                               name=f"attn{b % 2}")
                nc.vector.tensor_scalar_mul(attn[:], expE[:], rden[:])
                tpa = psp.tile([S, T], F32, tag=f"tp{b % 2}",
                               name=f"tp{b % 2}", bufs=2)
                nc.tensor.transpose(tpa[:], attn[:], ident[0:T, 0:T])
                atsb = pe.tile([S, T], BF16, tag=f"atsb{b % 2}",
                               name=f"atsb{b % 2}")
                nc.vector.tensor_copy(atsb[:], tpa[:])
                for m in range(8):
                    psc = gtile(m % 2, [128, T])
                    nc.tensor.matmul(
                        psc[:], encb[:, 1024 * b + 128 * m:1024 * b + 128 * (m + 1)],
                        atsb[:], start=True, stop=True)
                    if m % 2 == 0:
                        nc.scalar.copy(ctxT[m][:, bsl], psc[:])
                    else:
                        nc.vector.tensor_copy(ctxT[m][:, bsl], psc[:])

            outflat = OUT.ap().rearrange("b t d -> (b t) d")
            lhs_all = ctxT + sT + [ones]
            wt_all = [woutt[:, 512 * k:512 * (k + 1)] for k in range(12)] \
                + [woutb[:]]
            for mc in range(2):
                msl = slice(128 * mc, 128 * (mc + 1))
                ps = gtile(mc, [128, D])
                for k in range(13):
                    nc.tensor.matmul(
                        ps[:], lhs_all[k][:, msl], wt_all[k],
                        start=(k == 0), stop=(k == 12))
                osb = pa.tile([128, D], F32, tag=f"stg{mc}", name=f"osb{mc}")
                nc.scalar.activation(osb[:], ps[:], AF.Tanh)
                nc.sync.dma_start(outflat[msl, :], osb[:])

    nc.compile()
    return nc


def assemble(results):
    full = np.concatenate([r["out"] for r in results], axis=0)  # [B, T, D]
    outs = full.transpose(1, 0, 2)                              # [T, B, D]
    return np.ascontiguousarray(outs.reshape(-1, D).reshape(-1, T, D))


_nc_cache = None


def kernel(**inputs):
    global _nc_cache
    in_maps = host_prep(inputs)
    if _nc_cache is None:
        _nc_cache = build_program()
    res = run_bass_kernel_spmd(_nc_cache, in_maps, list(range(NCORES)))
    return assemble(res.results)


# revision 23
# speedup vs baseline: 2.2255x; 1.1303x over previous
"""Trainium2 Bass kernel for nn_Decoder_46042049413334.

Buggy 2-layer LSTM decoder with attention (B=32, T=64, S=128, D=512).

Structure (per core, batch sharded 8 ways, BS=4 examples/core):
  Phase A: xpart0 = [emb(tokens), 1] @ [W_ih0.T; b0]  -> XPsb0 (SBUF)
  Interleaved pass: layer-0 step t and layer-1 step t-2 run together;
    layer-1's xpart is accumulated per step directly into its gates
    PSUM from the transposed h2 history (hT0), so the PE stays busy
    enough to hold the HAM clock gate open (K=8/8).
  Phase E: attention + out-projection from hT1

Recurrence layout: gates PSUM [128, 512] where partition 32*j+b holds
(example b, d-block j) and the 512 free cols are {i,f,o,2g}x128 for
that d-block (g columns pre-scaled by 2 so tanh(g) = 2*sigmoid(2g)-1
comes out of a single full-width sigmoid). The four d-blocks' weight
streams run CONCURRENTLY in the PE array via tile_position=(0, 32*j)
column tiling. Elementwise runs once over all 128 partitions; c2 and
h2 are re-transposed per step ([128,128] PE transpose). Off-chain work
(tanh(c2), h2, its transpose/gather) is emitted one step late so no
engine FIFO ever blocks the recurrence chains.

Row ordering is b-major everywhere: row r = b_local*T + t.
"""
import numpy as np
import ml_dtypes
from contextlib import ExitStack

import concourse.bass as bass
import concourse.bacc as bacc
import concourse.tile as tile
from concourse import mybir, masks
from concourse.bass_utils import run_bass_kernel_spmd

F32 = mybir.dt.float32
BF16 = mybir.dt.bfloat16
AF = mybir.ActivationFunctionType
NPBF = ml_dtypes.bfloat16

B, T, S, D, L, V = 32, 64, 128, 512, 2, 32000
G = 4 * D        # 2048
DS = 2 * D       # 1024
NCORES = 8
BS = B // NCORES  # 4
R = BS * T        # 256 rows per core
LAG = 2          # layer-1 recurrence lag behind layer 0


# ---------------------------------------------------------------- host side

def _gate_perm():
    perm = np.zeros(G, dtype=np.int64)
    base = {0: 0, 1: 512, 2: 1536, 3: 1024}  # i, f, o, g
    for j in range(G):
        nb, pos = divmod(j, 512)
        sub, dd = divmod(pos, 128)
        perm[j] = base[sub] + nb * 128 + dd
    return perm


def host_prep(inputs):
    """Build the 8 per-core input maps (layout/gather work only)."""
    perm = _gate_perm()
    tokens = np.asarray(inputs["prev_tgt_tokens"])
    embed = np.asarray(inputs["embed"], dtype=np.float32)
    enc = np.asarray(inputs["encoder_out"], dtype=np.float32)
    mask = np.asarray(inputs["src_mask"])
    hid = np.asarray(inputs["hiddens"], dtype=np.float32)
    cells = np.asarray(inputs["cells"], dtype=np.float32)
    W_ih = np.asarray(inputs["W_ih"], dtype=np.float32)
    W_hh = np.asarray(inputs["W_hh"], dtype=np.float32)
    b_ih = np.asarray(inputs["b_ih"], dtype=np.float32)
    b_hh = np.asarray(inputs["b_hh"], dtype=np.float32)
    W_in = np.asarray(inputs["W_in"], dtype=np.float32)
    b_in = np.asarray(inputs["b_in"], dtype=np.float32)
    W_out = np.asarray(inputs["W_out"], dtype=np.float32)
    b_out = np.asarray(inputs["b_out"], dtype=np.float32)

    def bf(x):
        return np.ascontiguousarray(x, dtype=NPBF)

    WIH = []
    WHH = []
    gscale = np.ones(G, np.float32)
    for nb in range(4):
        gscale[512 * nb + 384:512 * (nb + 1)] = 2.0   # tanh(g)=2*sig(2g)-1
    for l in range(L):
        wihT = W_ih[l].T[:, perm] * gscale
        biasrow = ((b_ih[l] + b_hh[l])[perm] * gscale)[None, :]
        WIH.append(bf(np.concatenate([wihT, biasrow], 0)))   # [513, 2048]
        WHH.append(bf(W_hh[l].T[:, perm] * gscale))          # [512, 2048]
    WINT = bf(W_in.T)                                        # [512, 1024]
    WOUTT = bf(np.concatenate([W_out.T, b_out[None, :]], 0))  # [1537, 512]

    # xpart0 injection selectors, one per u = t%8:
    # XPsb0 partition (4j+b)*8+u feeds gates row 32j+b
    einj8 = np.zeros((8, 128, 128), np.float32)
    for u in range(8):
        for j in range(4):
            for b in range(BS):
                einj8[u, (4 * j + b) * 8 + u, 32 * j + b] = 1.0
    einj8 = bf(einj8.transpose(1, 0, 2).reshape(128, 8 * 128))

    # block selector: e4blk[j, p] = 1 iff p // 32 == j
    e4 = np.zeros((4, 128), np.float32)
    for j in range(4):
        e4[j, 32 * j:32 * (j + 1)] = 1.0
    e4 = bf(e4)

    in_maps = []
    for core in range(NCORES):
        bsl = slice(core * BS, (core + 1) * BS)
        xe = embed[tokens[bsl]]                              # [BS, T, D]
        Xaug = np.concatenate(
            [xe.reshape(R, D), np.ones((R, 1), np.float32)], axis=1)
        # permute rows so phase-A store DMAs land partition-parallel:
        # new row b*64 + (t%8)*8 + t//8  <- (b, t)
        rperm = np.zeros(R, np.int64)
        for b in range(BS):
            for t in range(T):
                rperm[b * T + (t % 8) * 8 + t // 8] = b * T + t
        XT0 = bf(Xaug[rperm].T)                              # [513, 256]
        enc_c = np.ascontiguousarray(enc[bsl])               # [BS, 128, 1024]
        encT_c = np.swapaxes(enc_c, 1, 2)                    # [BS, 1024, 128]
        offs = np.einsum("bsd,d->bs", enc_c, b_in) + np.where(mask[bsl], -1e9, 0.0)
        offs_rep = np.ascontiguousarray(
            np.broadcast_to(offs[:, None, :], (BS, T, S)), dtype=np.float32)
        # initial c2T: c2t0[l, p, 32k+b] = hid[l, b, 128k+p]
        th = hid[:, bsl].reshape(L, BS, 4, 128).transpose(0, 3, 2, 1)  # [L,128,4,BS]
        c2t0 = np.zeros((L, 128, 4, 32), np.float32)
        c2t0[:, :, :, 0:BS] = th
        c2t0 = bf(c2t0.reshape(L, 128, 128))
        # cells in partition layout: cellsp[l, 32j+b, p] = cells[l, b, 128j+p]
        tc_ = cells[:, bsl].reshape(L, BS, 4, 128).transpose(0, 2, 1, 3)  # [L,4,BS,128]
        cellsp = np.zeros((L, 4, 32, 128), np.float32)
        cellsp[:, :, 0:BS, :] = tc_
        cellsp = bf(cellsp.reshape(L, 128, 128))
        in_maps.append({
            "xt0": XT0,
            "wih0": WIH[0], "whh0": WHH[0],
            "wih1": WIH[1], "whh1": WHH[1],
            "wint": WINT, "woutt": WOUTT,
            "enc": bf(enc_c), "enct": bf(encT_c), "offs": offs_rep,
            "c2t0": c2t0, "cellsp": cellsp,
            "ones1": np.ones((1, R), NPBF),
            "einj8": einj8, "e4blk": e4,
        })
    return in_maps


# ------------------------------------------------------------- device build

def build_program():
    nc = bacc.Bacc("TRN2", target_bir_lowering=False, debug=False)

    XT0 = nc.dram_tensor("xt0", [513, R], BF16, kind="ExternalInput")
    WIH0 = nc.dram_tensor("wih0", [513, G], BF16, kind="ExternalInput")
    WHH0 = nc.dram_tensor("whh0", [D, G], BF16, kind="ExternalInput")
    WIH1 = nc.dram_tensor("wih1", [513, G], BF16, kind="ExternalInput")
    WHH1 = nc.dram_tensor("whh1", [D, G], BF16, kind="ExternalInput")
    WINT = nc.dram_tensor("wint", [D, DS], BF16, kind="ExternalInput")
    WOUTT = nc.dram_tensor("woutt", [DS + D + 1, D], BF16, kind="ExternalInput")
    ENC = nc.dram_tensor("enc", [BS, S, DS], BF16, kind="ExternalInput")
    ENCT = nc.dram_tensor("enct", [BS, DS, S], BF16, kind="ExternalInput")
    OFFS = nc.dram_tensor("offs", [BS, T, S], F32, kind="ExternalInput")
    C2T0 = nc.dram_tensor("c2t0", [L, 128, 128], BF16, kind="ExternalInput")
    CELLSP = nc.dram_tensor("cellsp", [L, 128, 128], BF16, kind="ExternalInput")
    ONES1 = nc.dram_tensor("ones1", [1, R], BF16, kind="ExternalInput")
    EINJ = nc.dram_tensor("einj8", [128, 8 * 128], BF16, kind="ExternalInput")
    E4BLK = nc.dram_tensor("e4blk", [4, 128], BF16, kind="ExternalInput")
    OUT = nc.dram_tensor("out", [BS, T, D], F32, kind="ExternalOutput")

    with tile.TileContext(nc) as tc, ExitStack() as ctx:
        cpool = ctx.enter_context(tc.tile_pool(name="const", bufs=1))
        ident = cpool.tile([128, 128], F32)
        masks.make_identity(nc, ident[:])
        identb = cpool.tile([128, 128], BF16, name="identb")
        masks.make_identity(nc, identb[:])
        ones = cpool.tile([1, R], BF16, name="ones")
        nc.sync.dma_start(ones[:], ONES1.ap())
        einj8 = cpool.tile([128, 8 * 128], BF16, name="einj8")
        nc.sync.dma_start(einj8[:], EINJ.ap())
        e4blk = cpool.tile([4, 128], BF16, name="e4blk")
        nc.sync.dma_start(e4blk[:], E4BLK.ap())
        ones128 = cpool.tile([128, 128], BF16, name="ones128")
        nc.gpsimd.memset(ones128[:], 1.0)

        psp = ctx.enter_context(tc.tile_pool(name="ps", bufs=1, space="PSUM"))

        def gtile(idx, shape):
            return psp.tile(shape, F32, tag=f"g{idx}", name=f"g{idx}",
                            bufs=2 if idx < 2 else 1)

        # persistent SBUF xpart0:
        # XPsb0[(4*nb+b)*8 + t%8, (t//8)*512 + c] = xpart0[b,t,512nb+c]
        xpp = ctx.enter_context(tc.tile_pool(name="xps", bufs=1))
        XPsb0 = xpp.tile([128, (T // 8) * 512], BF16, name="xpsb0")

        # transposed h2 history per layer: hT[p, k*256 + b*64 + t]
        hT = [xpp.tile([128, 4 * R], BF16, name=f"hT{l}") for l in range(L)]

        # ---------------- Phase A inputs (packed, few DMAs) ----------------
        pa = ctx.enter_context(tc.tile_pool(name="pa", bufs=1))
        xtt = pa.tile([128, 4 * R], BF16, tag="xtt", name="xtt")
        nc.sync.dma_start(
            xtt[:].rearrange("p (k c) -> p k c", k=4),
            XT0.ap()[0:512].rearrange("(k p) c -> p k c", k=4))
        xt4 = pa.tile([1, R], BF16, tag="xt4", name="xt4")
        nc.sync.dma_start(xt4[:], XT0.ap()[512:513, :])
        wkt = pa.tile([128, 4 * G], BF16, tag="wkt", name="wkt")
        nc.sync.dma_start(
            wkt[:].rearrange("p (k c) -> p k c", k=4),
            WIH0.ap()[0:512].rearrange("(k p) c -> p k c", k=4))
        wk4 = pa.tile([1, G], BF16, tag="wk4", name="wk4")
        nc.sync.dma_start(wk4[:], WIH0.ap()[512:513, :])

        # PE warm-up: dummy matmuls on the identity while DMAs land
        wps = psp.tile([128, 128], F32, tag="g0", name="g0", bufs=2)
        for w in range(48):
            nc.tensor.matmul(wps[:], identb[:], identb[:],
                             start=True, stop=True, skip_group_check=True)

        # prefetch pool: recurrence weights + attention operands
        pf = ctx.enter_context(tc.tile_pool(name="pf", bufs=1))
        cTb = pf.tile([128, 2 * 128], BF16, tag="cTb", name="cTb")
        nc.sync.dma_start(
            cTb[:].rearrange("p (l c) -> p l c", l=2),
            CELLSP.ap().rearrange("l p c -> p l c"))
        c2T0b = pf.tile([128, 2 * 128], BF16, tag="c2T0b", name="c2T0b")
        nc.sync.dma_start(
            c2T0b[:].rearrange("p (l c) -> p l c", l=2),
            C2T0.ap().rearrange("l p c -> p l c"))
        whht = []
        for l, Wd in ((0, WHH0), (1, WHH1)):
            wt = pf.tile([128, 4 * G], BF16, tag=f"whh{l}", name=f"whh{l}")
            nc.sync.dma_start(
                wt[:].rearrange("p (k c) -> p k c", k=4),
                Wd.ap().rearrange("(k p) c -> p k c", k=4))
            whht.append(wt)
        wihC = pf.tile([128, 4 * G], BF16, tag="wihC", name="wihC")
        nc.sync.dma_start(
            wihC[:].rearrange("p (k c) -> p k c", k=4),
            WIH1.ap()[0:512].rearrange("(k p) c -> p k c", k=4))
        wihC4 = pf.tile([4, 512], BF16, tag="wihC4", name="wihC4")
        nc.sync.dma_start(wihC4[:], WIH1.ap()[512:513, :].rearrange(
            "a (j c) -> (a j) c", j=4))
        # ---------------- Phase A: xpart0 ----------------
        for mc in range(2):
            for nb in range(4):
                ps = gtile(nb % 2, [128, 512])
                for k in range(4):
                    nc.tensor.matmul(
                        ps[:],
                        xtt[:, 256 * k + 128 * mc:256 * k + 128 * (mc + 1)],
                        wkt[:, 2048 * k + 512 * nb:2048 * k + 512 * (nb + 1)],
                        start=(k == 0), stop=False)
                nc.tensor.matmul(
                    ps[:], xt4[:, 128 * mc:128 * (mc + 1)],
                    wk4[:, 512 * nb:512 * (nb + 1)],
                    start=False, stop=True)
                sb = pa.tile([128, 512], BF16, tag=f"stg{nb}", name=f"stg{nb}")
                nc.scalar.copy(sb[:], ps[:])
                p0 = (4 * nb + 2 * mc) * 8
                dst = XPsb0[p0:p0 + 16, :].rearrange(
                    "p (q c) -> p q c", c=512)
                nc.sync.dma_start(dst, sb[:])

        # ---------------- Interleaved recurrence passes ----------------
        rp = ctx.enter_context(tc.tile_pool(name="rp", bufs=2))

        def linit(l):
            return {"l": l, "cT": cTb[:, 128 * l:128 * (l + 1)],
                    "c2T": c2T0b[:, 128 * l:128 * (l + 1)], "whh": whht[l],
                    "sall_prev": None, "c2h_prev": None, "tprev": -1}

        def lstep_mm(st, t):
            """Inject/xpart + W_hh rounds for step t (PE bulk)."""
            l = st["l"]
            gates = gtile(l, [128, 512])
            if l == 0:
                rhs = XPsb0[:, 512 * (t // 8):512 * (t // 8 + 1)]
                nc.tensor.matmul(
                    gates[:], einj8[:, 128 * (t % 8):128 * (t % 8 + 1)], rhs,
                    start=True, stop=False, skip_group_check=True)
            else:
                # xpart1(t) accumulated in place: bias row first (writes all
                # 128 partitions), then W_ih1 rounds from hT0 columns of t
                nc.tensor.matmul(gates[:], e4blk[:], wihC4[:],
                                 start=True, stop=False, skip_group_check=True)
                for k in range(4):
                    lhsT = hT[0][:].rearrange(
                        "p (k b t) -> p k b t", k=4, b=BS)[:, k, :, t]
                    for j in range(4):
                        nc.tensor.matmul(
                            gates[32 * j:32 * j + BS, :],
                            lhsT,
                            wihC[:, 2048 * k + 512 * j:2048 * k + 512 * (j + 1)],
                            start=False, stop=False,
                            tile_position=(0, 32 * j), skip_group_check=True)
            for k in range(4):
                lhsT = st["c2T"][:, 32 * k:32 * k + BS]
                for j in range(4):
                    nc.tensor.matmul(
                        gates[32 * j:32 * j + BS, :],
                        lhsT,
                        st["whh"][:, 2048 * k + 512 * j:2048 * k + 512 * (j + 1)],
                        start=False, stop=(k == 3),
                        tile_position=(0, 32 * j), skip_group_check=True)
            st["gates"] = gates

        def lstep_sigma(st, t):
            """sigma + c2 elementwise chain for step t (no transpose)."""
            l = st["l"]
            gates = st["gates"]
            sall = rp.tile([128, 512], F32, tag=f"sa{l}", name=f"sa{l}",
                           bufs=3)
            nc.scalar.activation(sall[:], gates[:], AF.Sigmoid)
            m1 = rp.tile([128, 128], BF16, tag=f"m1{l}", name=f"m1{l}")
            nc.gpsimd.tensor_mul(m1[:], sall[:, 128:256], st["cT"])
            tgv = rp.tile([128, 128], BF16, tag=f"tg{l}", name=f"tg{l}")
            nc.vector.scalar_tensor_tensor(
                tgv[:], sall[:, 384:512], 2.0, ones128[:],
                mybir.AluOpType.mult, mybir.AluOpType.subtract)
            m2 = rp.tile([128, 128], BF16, tag=f"m2{l}", name=f"m2{l}")
            nc.vector.tensor_mul(m2[:], sall[:, 0:128], tgv[:])
            c2h = rp.tile([128, 128], BF16, tag=f"c2h{l}", name=f"c2h{l}",
                          bufs=3)
            nc.vector.tensor_add(c2h[:], m1[:], m2[:])
            st["sall"] = sall
            st["c2h"] = c2h

        def lstep_transpose(st):
            """c2 transpose + cast — emitted at a PE-queue position where
            the chain dependency (add) has already resolved."""
            l = st["l"]
            tp = psp.tile([128, 128], BF16, tag=f"tp{l}", name=f"tp{l}",
                          bufs=2)
            nc.tensor.transpose(tp[:], st["c2h"][:], identb[:])
            c2T_new = rp.tile([128, 128], BF16, tag=f"c2T{l}", name=f"c2T{l}")
            nc.vector.tensor_copy(c2T_new[:], tp[:])
            st["c2T"] = c2T_new

        def lstep_branch(st):
            """Delayed h2 branch for the PREVIOUS step (never blocks chains)."""
            l = st["l"]
            if st["sall_prev"] is not None:
                tc2 = rp.tile([128, 128], BF16, tag=f"tc2{l}", name=f"tc2{l}")
                nc.scalar.activation(tc2[:], st["c2h_prev"][:], AF.Tanh)
                h2 = rp.tile([128, 128], BF16, tag=f"h2{l}", name=f"h2{l}")
                nc.gpsimd.tensor_mul(h2[:], st["sall_prev"][:, 256:384], tc2[:])
                tp2 = psp.tile([128, 128], BF16, tag=f"tp{l}", name=f"tp{l}",
                               bufs=2)
                nc.tensor.transpose(tp2[:], h2[:], identb[:])
                src_ = tp2[:].rearrange("p (k r) -> p k r", k=4)[:, :, 0:BS]
                dst = hT[l][:].rearrange(
                    "p (k b t) -> p k b t", k=4, b=BS)[:, :, :, st["tprev"]]
                nc.vector.tensor_copy(dst, src_)
            st["sall_prev"] = st["sall"]
            st["c2h_prev"] = st["c2h"]
            st["tprev"] = st["tprev"] + 1

        # phase-E operands: transferred during the recurrence pass
        # phase-E operands: transferred during the recurrence pass
        wint = pf.tile([128, 4 * DS], BF16, tag="wint", name="wint")
        nc.sync.dma_start(
            wint[:].rearrange("p (k c) -> p k c", k=4),
            WINT.ap().rearrange("(k p) c -> p k c", k=4))
        encb = pf.tile([S, 4 * DS], BF16, tag="encb", name="encb")
        nc.sync.dma_start(
            encb[:].rearrange("p (b c) -> p b c", b=BS),
            ENC.ap().rearrange("b s d -> s b d"))
        enctb = pf.tile([128, BS * 8 * S], BF16, tag="enctb", name="enctb")
        nc.sync.dma_start(
            enctb[:].rearrange("p (b k s) -> p b k s", b=BS, k=8),
            ENCT.ap().rearrange("b (k p) s -> p b k s", k=8))
        offsb = pf.tile([T, BS * S], F32, tag="offsb", name="offsb")
        nc.sync.dma_start(
            offsb[:].rearrange("p (b s) -> p b s", b=BS),
            OFFS.ap().rearrange("b t s -> t b s"))
        woutt = pf.tile([128, 12 * D], BF16, tag="woutt", name="woutt")
        nc.sync.dma_start(
            woutt[:].rearrange("p (k c) -> p k c", k=12),
            WOUTT.ap()[0:1536].rearrange("(k p) c -> p k c", k=12))
        woutb = pf.tile([1, D], BF16, tag="woutb", name="woutb")
        nc.sync.dma_start(woutb[:], WOUTT.ap()[1536:1537, :])

        st0 = linit(0)
        st1 = linit(1)
        for ss in range(T + LAG + 1):
            if ss < T:
                lstep_mm(st0, ss)                # PE: inj + rounds L0(t)
            if LAG < ss <= T + LAG:
                lstep_transpose(st1)             # PE: T_c2 L1(t'-1) (ready)
            if ss < T:
                lstep_sigma(st0, ss)
            if LAG <= ss < T + LAG:
                lstep_mm(st1, ss - LAG)          # PE: bias/xpart/whh L1(t')
            if LAG < ss <= T + LAG:
                lstep_branch(st1)                # PE: T_h2 L1(t'-1)
            if LAG <= ss < T + LAG:
                lstep_sigma(st1, ss - LAG)
            if ss < T:
                lstep_transpose(st0)             # PE: T_c2 L0(t) (ready)
            if ss <= T:
                lstep_branch(st0)                # PE: T_h2 L0(t-1)
        lstep_branch(st1)                        # flush T_h2 L1(63)

        # ---------------- Phase E: attention + out proj ----------------
        wkt2 = pa.tile([128, 4 * G], BF16, tag="wkt", name="wkt2")
        with tc.tile_pool(name="pe", bufs=1) as pe:
            sT = [hT[1][:, 256 * k:256 * (k + 1)] for k in range(4)]

            xqT = []
            for m in range(8):
                ps = gtile(m % 2, [128, R])
                for k in range(4):
                    nc.tensor.matmul(
                        ps[:], wint[:, 1024 * k + 128 * m:1024 * k + 128 * (m + 1)],
                        sT[k], start=(k == 0), stop=(k == 3))
                xq = wkt2[:, 256 * m:256 * (m + 1)]
                if m % 2 == 0:
                    nc.scalar.copy(xq, ps[:])
                else:
                    nc.vector.tensor_copy(xq, ps[:])
                xqT.append(xq)

            ctxT = [wkt2[:, 2048 + 256 * m:2048 + 256 * (m + 1)]
                    for m in range(8)]
            for b in range(BS):
                bsl = slice(T * b, T * (b + 1))
                eps = psp.tile([T, S], F32, tag=f"g{b % 2}",
                               name=f"g{b % 2}", bufs=2)
                for k in range(8):
                    nc.tensor.matmul(
                        eps[:], xqT[k][:, bsl],
                        enctb[:, 1024 * b + 128 * k:1024 * b + 128 * (k + 1)],
                        start=(k == 0), stop=(k == 7))
                esb = pe.tile([T, S], F32, tag=f"esb{b % 2}",
                              name=f"esb{b % 2}")
                nc.vector.tensor_add(esb[:], eps[:],
                                     offsb[:, 128 * b:128 * (b + 1)])
                negmax = pe.tile([T, 1], F32, tag=f"negmax{b % 2}",
                                 name=f"negmax{b % 2}")
                nc.vector.reduce_max(
                    negmax[:], esb[:], axis=mybir.AxisListType.X, negate=True)
                expE = pe.tile([T, S], F32, tag=f"expE{b % 2}",
                               name=f"expE{b % 2}")
                den = pe.tile([T, 1], F32, tag=f"den{b % 2}",
                              name=f"den{b % 2}")
                nc.scalar.activation(
                    expE[:], esb[:], AF.Exp, bias=negmax[:], accum_out=den[:])
                rden = pe.tile([T, 1], F32, tag=f"rden{b % 2}",
                               name=f"rden{b % 2}")
                nc.vector.reciprocal(rden[:], den[:])
                attn = pe.tile([T, S], F32, tag=f"attn{b % 2}",
                               name=f"attn{b % 2}")
                nc.vector.tensor_scalar_mul(attn[:], expE[:], rden[:])
                tpa = psp.tile([S, T], F32, tag=f"tp{b % 2}",
                               name=f"tp{b % 2}", bufs=2)
                nc.tensor.transpose(tpa[:], attn[:], ident[0:T, 0:T])
                atsb = pe.tile([S, T], BF16, tag=f"atsb{b % 2}",
                               name=f"atsb{b % 2}")
                nc.vector.tensor_copy(atsb[:], tpa[:])
                for m in range(8):
                    psc = gtile(m % 2, [128, T])
                    nc.tensor.matmul(
                        psc[:], encb[:, 1024 * b + 128 * m:1024 * b + 128 * (m + 1)],
                        atsb[:], start=True, stop=True)
                    if m % 2 == 0:
                        nc.scalar.copy(ctxT[m][:, bsl], psc[:])
                    else:
                        nc.vector.tensor_copy(ctxT[m][:, bsl], psc[:])

            outflat = OUT.ap().rearrange("b t d -> (b t) d")
            lhs_all = ctxT + sT + [ones]
            wt_all = [woutt[:, 512 * k:512 * (k + 1)] for k in range(12)] \
                + [woutb[:]]
            for mc in range(2):
                msl = slice(128 * mc, 128 * (mc + 1))
                ps = gtile(mc, [128, D])
                for k in range(13):
                    nc.tensor.matmul(
                        ps[:], lhs_all[k][:, msl], wt_all[k],
                        start=(k == 0), stop=(k == 12))
                osb = pa.tile([128, D], F32, tag=f"stg{mc}", name=f"osb{mc}")
                nc.scalar.activation(osb[:], ps[:], AF.Tanh)
                nc.sync.dma_start(outflat[msl, :], osb[:])

    nc.compile()
    return nc


def assemble(results):
    full = np.concatenate([r["out"] for r in results], axis=0)  # [B, T, D]
    outs = full.transpose(1, 0, 2)                              # [T, B, D]
    return np.ascontiguousarray(outs.reshape(-1, D).reshape(-1, T, D))


_nc_cache = None


def kernel(**inputs):
    global _nc_cache
    in_maps = host_prep(inputs)
    if _nc_cache is None:
        _nc_cache = build_program()
    res = run_bass_kernel_spmd(_nc_cache, in_maps, list(range(NCORES)))
    return assemble(res.results)
